# revision 19
# baseline (speedup 1.0000x reference)
"""VQ codebook encoding kernel for Trainium2, sharded over 8 NeuronCores.

Math (per shard of N tokens):
    l2[n,k]  = ||x_n - c_k||            (NOT squared)
    W        = softmax_k(l2 * scale_k)
    E[k,d]   = sum_n W[n,k] * (x[n,d] - c[k,d])
             = (W^T X)[k,d] - S_k * c[k,d],   S_k = sum_n W[n,k]

so we never materialize the (N,K,D) residual tensor.  The N axis is
sharded 8 ways; each core returns a partial (K,D) E which the host sums.

Layout tricks:
- x is fed twice per core: natural [n,D] rows (aggregation matmul; a
  [1,0] pad per 128-token tile makes one matmul yield W^T[X|1|0] =
  [M | S | 0] and keeps free dims even for fp32r) and host-pre-transposed
  [D,n] (score matmuls).  Host transposes are free and kill all on-chip
  transposes.
- ||x_n||^2: square the [x|1|0] tile wholesale (ACT) + segmented reduce
  (DVE); the ones column contributes +1, folded into the (cc-1) const.
- softmax: logits = l2*s = exp(0.5*(ln(l2^2) + ln(s^2))); hw_specs'
  cached activation-table map is seeded so ln/exp/square all resolve to
  the one table set containing all three -> exactly one ACT_TABLE_LOAD.
- matmuls run float32r (single-pass on the PE) -- fp32 is 2-pass.
- per-k constants (cc-1, s, ln s^2) are fed as [128,32] tiles and read
  with stride-0 APs to broadcast across the 16 token-tiles; DVE free-dim
  step-0 broadcasts also splice ||x||^2 and 1/den per token-tile.
- DMA order is chosen so the score-matmul operands land first (HWDGE
  transfers complete in FIFO order).
"""

import sys

if "/opt/trn_rl_repo" not in sys.path:
    sys.path.insert(0, "/opt/trn_rl_repo")

import os as _os

import numpy as np

N_CORES = 8
N, K, D = 16384, 32, 128
NPC = N // N_CORES          # tokens per core = 2048
NT = NPC // 128             # 128-token tiles per core = 16
SC = 2                      # superchunks
TPS = NT // SC              # tiles per superchunk = 8
XNW = 130                   # x tile width incl. ones col + pad (fp32r wants even)

_CACHE = {}

_MMDT = _os.environ.get("KMMDT", "f32r")            # f32r | f32
_RAW = _os.environ.get("KRAW", "1") == "1"          # raw bass vs TileContext


def _force_combined_act_table(nc, mybir):
    """Seed hw_specs' cached activation-table dict so the ln/exp/square
    activations all resolve to the one set that contains all three
    ("natural_log_exp_and_others"), giving a single ACT_TABLE_LOAD instead
    of per-func table thrash.  Only mutates the per-process cache copy;
    set ids stay aligned with the compiler's act_info.json."""
    import concourse.hw_specs as hw_specs

    AFT = mybir.ActivationFunctionType
    tables = hw_specs.get_activation_tables(nc.m.arch)
    if "natural_log_exp_and_others" not in tables:
        return
    for name, funcs in tables.items():
        if name != "natural_log_exp_and_others":
            funcs.discard(AFT.Exp)
            funcs.discard(AFT.Ln)
            funcs.discard(AFT.Square)


def _build_nc():
    import concourse.bacc as bacc
    import concourse.bass as bass
    import concourse.mybir as mybir
    from concourse.tile import TileContext

    f32 = mybir.dt.float32
    f32r = mybir.dt.float32r
    AFT = mybir.ActivationFunctionType
    ALU = mybir.AluOpType

    mmdt = f32r if _MMDT == "f32r" else f32

    nc = bacc.Bacc(None, target_bir_lowering=False)
    _force_combined_act_table(nc, mybir)

    xn = nc.dram_tensor("xn", [128, NT * XNW], mmdt, kind="ExternalInput")
    xtp = nc.dram_tensor("xtp", [128, NPC], mmdt, kind="ExternalInput")
    ct2 = nc.dram_tensor("ct2", [128, K], mmdt, kind="ExternalInput")
    # packed per-k consts: [cc-1 | s | ln s^2], each [128, K]
    cst = nc.dram_tensor("cst", [128, 3 * K], f32, kind="ExternalInput")
    codes_in = nc.dram_tensor("codes", [K, D], f32, kind="ExternalInput")
    e_out = nc.dram_tensor("E", [K, D], f32, kind="ExternalOutput")

    def fcast(ap):
        # view an mmdt tile as plain f32 for non-matmul consumers
        return ap.bitcast(f32) if mmdt is f32r else ap

    def bcast_t(ap32, count):
        # [128, 32] const -> [128, count, 32] via a stride-0 middle dim
        return bass.AP(
            tensor=ap32.tensor,
            offset=ap32.offset,
            ap=[list(ap32.ap[0]), [0, count], list(ap32.ap[1])],
        )

    def bcast_k(apw, count):
        # [128, w] per-tile scalars -> [128, w, count] via stride-0 inner dim
        return bass.AP(
            tensor=apw.tensor,
            offset=apw.offset,
            ap=[list(apw.ap[0]), list(apw.ap[1]), [0, count]],
        )

    with TileContext(nc) as tc:
        with (
            tc.tile_pool(name="singles", bufs=1) as singles,
            tc.tile_pool(name="data", bufs=SC) as data,
            tc.tile_pool(name="work", bufs=SC) as work,
            tc.tile_pool(name="psum_sc", bufs=SC, space="PSUM") as psum_sc,
            tc.tile_pool(name="psum_ag", bufs=1, space="PSUM") as psum_ag,
        ):
            # ---- tiny consts first (they gate everything) ----
            ct2_sb = singles.tile([128, K], mmdt)
            nc.sync.dma_start(out=ct2_sb, in_=ct2[:, :])
            cst_sb = singles.tile([128, 3 * K], f32)
            nc.sync.dma_start(out=cst_sb, in_=cst[:, :])
            ccb_sb = cst_sb[:, 0:K]
            sclb_sb = cst_sb[:, K : 2 * K]
            lns2_sb = cst_sb[:, 2 * K : 3 * K]

            # ---- data DMAs, in the order compute needs them ----
            xn_chunks = []
            xtp_chunks = []
            for c in range(SC):
                xn_c = data.tile([128, TPS * XNW], mmdt, tag="xn")
                xtp_c = data.tile([128, TPS * 128], mmdt, tag="xtp")
                xn_chunks.append(xn_c)
                xtp_chunks.append(xtp_c)
            # first half of xtp0 lands first so matmuls can start asap
            h = TPS * 128 // 2
            nc.sync.dma_start(out=xtp_chunks[0][:, :h], in_=xtp[:, :h])
            nc.sync.dma_start(out=xtp_chunks[0][:, h:], in_=xtp[:, h : TPS * 128])
            nc.sync.dma_start(out=xn_chunks[0], in_=xn[:, : TPS * XNW])
            nc.sync.dma_start(out=xtp_chunks[1], in_=xtp[:, TPS * 128 : 2 * TPS * 128])
            nc.sync.dma_start(out=xn_chunks[1], in_=xn[:, TPS * XNW : 2 * TPS * XNW])
            codes_sb = singles.tile([K, D], f32)
            nc.sync.dma_start(out=codes_sb, in_=codes_in[:, :])

            psum_ms = psum_ag.tile([K, XNW], f32)
            w_chunks = []

            # ---- phase A: scores + softmax weights, per superchunk ----
            for c in range(SC):
                xn_c = xn_chunks[c]
                xtp_c = xtp_chunks[c]

                # scores: -2 x.c for 8 tiles into one PSUM bank
                ps_c = psum_sc.tile([128, TPS * K], f32)
                for i in range(TPS):
                    nc.tensor.matmul(
                        ps_c[:, i * K : (i + 1) * K],
                        xtp_c[:, i * 128 : (i + 1) * 128],
                        ct2_sb,
                        start=True,
                        stop=True,
                    )

                # xx+1 per (token, tile): square the whole [x|1|0] superchunk
                scr_c = work.tile([128, TPS * XNW], f32, tag="scr")
                nc.scalar.activation(out=scr_c, in_=fcast(xn_c), func=AFT.Square)
                xxp_c = work.tile([128, TPS], f32, tag="xxp")
                nc.vector.tensor_reduce(
                    out=xxp_c,
                    in_=scr_c.rearrange("p (t w) -> p t w", w=XNW),
                    axis=mybir.AxisListType.X,
                    op=ALU.add,
                )

                # ccxx[p,t,k] = (cc_k-1) + (xx+1)[p,t]  (independent of PE)
                ccxx_c = work.tile([128, TPS * K], f32, tag="ccxx")
                ccxx3 = ccxx_c.rearrange("p (t k) -> p t k", k=K)
                nc.vector.tensor_add(ccxx3, bcast_t(ccb_sb, TPS), bcast_k(xxp_c, K))

                # A = l2^2 = -2xc + ccxx
                a_c = work.tile([128, TPS * K], f32, tag="a")
                nc.vector.tensor_add(a_c, ps_c, ccxx_c)

                # logits = l2*s = exp(0.5*(ln(l2^2) + ln(s^2)))
                l_c = work.tile([128, TPS * K], f32, tag="l")
                nc.scalar.activation(out=l_c, in_=a_c, func=AFT.Ln)
                t_c = work.tile([128, TPS * K], f32, tag="t")
                nc.vector.tensor_add(
                    t_c.rearrange("p (t k) -> p t k", k=K),
                    l_c.rearrange("p (t k) -> p t k", k=K),
                    bcast_t(lns2_sb, TPS),
                )
                p_c = work.tile([128, TPS * K], f32, tag="p")
                nc.scalar.activation(out=p_c, in_=t_c, func=AFT.Exp, scale=0.5)
                # EW = exp(logits)
                ew_c = work.tile([128, TPS * K], f32, tag="ew")
                nc.scalar.activation(out=ew_c, in_=p_c, func=AFT.Exp)

                # denominators + reciprocal + normalize
                den_c = work.tile([128, TPS], f32, tag="den")
                nc.vector.tensor_reduce(
                    out=den_c,
                    in_=ew_c.rearrange("p (t k) -> p t k", k=K),
                    axis=mybir.AxisListType.X,
                    op=ALU.add,
                )
                rden_c = work.tile([128, TPS], f32, tag="rden")
                nc.vector.reciprocal(out=rden_c, in_=den_c)

                w_c = work.tile([128, TPS * K], mmdt, tag="w")
                nc.vector.tensor_mul(
                    w_c.rearrange("p (t k) -> p t k", k=K),
                    ew_c.rearrange("p (t k) -> p t k", k=K),
                    bcast_k(rden_c, K),
                )
                w_chunks.append(w_c)

            # ---- phase B: aggregation matmuls ----
            for c in range(SC):
                for i in range(TPS):
                    t = c * TPS + i
                    nc.tensor.matmul(
                        psum_ms,
                        w_chunks[c][:, i * K : (i + 1) * K],
                        xn_chunks[c][:, i * XNW : (i + 1) * XNW],
                        start=(t == 0),
                        stop=(t == NT - 1),
                    )

            # ---- final: E = M - S * codes ----
            s_sb = singles.tile([K, 1], f32)
            nc.vector.tensor_copy(s_sb, psum_ms[:, 128:129])
            tmp = singles.tile([K, D], f32)
            nc.vector.tensor_scalar_mul(tmp, codes_sb, s_sb)
            e_sb = singles.tile([K, D], f32)
            nc.vector.tensor_sub(e_sb, psum_ms[:, 0:128], tmp)
            nc.sync.dma_start(out=e_out[:, :], in_=e_sb)

    nc.compile()
    return nc




def _build_nc_raw():
    """Raw-bass variant: same pipeline as the Tile builder but with
    hand-rolled semaphores and no TileContext, which drops the ~10us
    end-of-kernel drain+barrier teardown and most per-op sync overhead.
    DMA dispatches are split across the two HWDGE queues (sync + scalar)."""
    import concourse.bacc as bacc
    import concourse.bass as bass
    import concourse.mybir as mybir

    f32 = mybir.dt.float32
    f32r = mybir.dt.float32r
    AFT = mybir.ActivationFunctionType
    ALU = mybir.AluOpType

    mmdt = f32r if _MMDT == "f32r" else f32

    nc = bacc.Bacc(None, target_bir_lowering=False)
    _force_combined_act_table(nc, mybir)

    HS = TPS * XNW                 # xn elems per superchunk (1040)
    HT = TPS * 128                 # xtp elems per superchunk (1024)

    xn = nc.dram_tensor("xn", [128, NT * XNW], mmdt, kind="ExternalInput")
    xtp = nc.dram_tensor("xtp", [128, NPC], mmdt, kind="ExternalInput")
    cstr = nc.dram_tensor("cstr", [128, K], mmdt, kind="ExternalInput")   # ct2
    cst = nc.dram_tensor("cst", [128, 2 * K], f32, kind="ExternalInput")  # cc-1|lns2
    codes_in = nc.dram_tensor("codes", [K, D], f32, kind="ExternalInput")
    e_out = nc.dram_tensor("E", [K, D], f32, kind="ExternalOutput")

    sb = lambda name, shape, dt: nc.alloc_sbuf_tensor(name, shape, dt)
    ct2_sb = sb("ct2_sb", [128, K], mmdt)
    cst_sb = sb("cst_sb", [128, 2 * K], f32)
    xtp_sb = sb("xtp_sb", [128, NPC], mmdt)
    xn_sb = sb("xn_sb", [128, NT * XNW], mmdt)
    codes_sb = sb("codes_sb", [K, D], f32)
    scr = [sb(f"scr{c}", [128, HS], f32) for c in range(SC)]
    xxp = [sb(f"xxp{c}", [128, TPS], f32) for c in range(SC)]
    ccxx = [sb(f"ccxx{c}", [128, TPS * K], f32) for c in range(SC)]
    a_t = [sb(f"a{c}", [128, TPS * K], f32) for c in range(SC)]
    l_t = [sb(f"l{c}", [128, TPS * K], f32) for c in range(SC)]
    t_t = [sb(f"t{c}", [128, TPS * K], f32) for c in range(SC)]
    p_t = [sb(f"p{c}", [128, TPS * K], f32) for c in range(SC)]
    ew_t = [sb(f"ew{c}", [128, TPS * K], f32) for c in range(SC)]
    den = [sb(f"den{c}", [128, TPS], f32) for c in range(SC)]
    rden = [sb(f"rden{c}", [128, TPS], f32) for c in range(SC)]
    w_t = [sb(f"w{c}", [128, TPS * K], mmdt) for c in range(SC)]
    s_sb = sb("s_sb", [K, 1], f32)
    tmp_sb = sb("tmp_sb", [K, D], f32)
    e_sb = sb("e_sb", [K, D], f32)

    # full-bank allocations so PE writes and DVE reads never share a bank
    ps = [nc.alloc_psum_tensor(f"ps{c}", [128, 512], f32) for c in range(SC)]
    pms = nc.alloc_psum_tensor("pms", [K, XNW], f32)

    ct2v = ct2_sb[:, :]
    ccbv = cst_sb[:, 0:K]
    lnsv = cst_sb[:, K : 2 * K]

    def fc(ap):
        return ap.bitcast(f32) if mmdt is f32r else ap

    def bct(ap32, count):
        return bass.AP(
            tensor=ap32.tensor,
            offset=ap32.offset,
            ap=[list(ap32.ap[0]), [0, count], list(ap32.ap[1])],
        )

    def bck(apw, count):
        return bass.AP(
            tensor=apw.tensor,
            offset=apw.offset,
            ap=[list(apw.ap[0]), list(apw.ap[1]), [0, count]],
        )

    def t3(ap, k=K):
        return ap.rearrange("p (t k) -> p t k", k=k)

    sQr = nc.alloc_semaphore("sQr")      # ct2 DMA done
    sQc = nc.alloc_semaphore("sQc")      # cst DMA done
    sQ1 = nc.alloc_semaphore("sQ1")      # xtp first half
    sQ2 = nc.alloc_semaphore("sQ2")      # xtp second half
    sQ3 = nc.alloc_semaphore("sQ3")      # xtp sc1
    aQ0 = nc.alloc_semaphore("aQ0")      # xn sc0
    aQ1 = nc.alloc_semaphore("aQ1")      # xn sc1
    aQ2 = nc.alloc_semaphore("aQ2")      # codes
    mmS = nc.alloc_semaphore("mmS")      # PE: score matmuls done per sc
    aggS = nc.alloc_semaphore("aggS")    # PE: aggregation done
    sqS = nc.alloc_semaphore("sqS")      # ACT squares done per sc
    lnN = nc.alloc_semaphore("lnN")      # ACT ln done per sc
    ewN = nc.alloc_semaphore("ewN")      # ACT exp(exp) done per sc
    dvA = nc.alloc_semaphore("dvA")      # DVE A=l2^2 ready per sc
    dvT = nc.alloc_semaphore("dvT")      # DVE t=ln(l2^2 s^2) ready per sc
    wR = nc.alloc_semaphore("wR")        # DVE W ready per sc
    eR = nc.alloc_semaphore("eR")        # E ready in SBUF
    oD = nc.alloc_semaphore("oD")        # output DMA done

    with nc.Block(no_gpsimd_drain=True) as block:

        @block.sync
        def _(sync):
            sync.dma_start(out=ct2_sb[:, :], in_=cstr[:, :]).then_inc(sQr, 16)
            sync.dma_start(out=cst_sb[:, :], in_=cst[:, :]).then_inc(sQc, 16)
            h = HT // 2
            sync.dma_start(out=xtp_sb[:, :h], in_=xtp[:, :h]).then_inc(sQ1, 16)
            sync.dma_start(out=xtp_sb[:, h:HT], in_=xtp[:, h:HT]).then_inc(sQ2, 16)
            sync.dma_start(out=xtp_sb[:, HT:], in_=xtp[:, HT:]).then_inc(sQ3, 16)
            sync.wait_ge(eR, 1)
            sync.dma_start(out=e_out[:, :], in_=e_sb[:, :]).then_inc(oD, 16)
            sync.wait_ge(oD, 16)

        @block.scalar
        def _(scalar):
            # dispatch the n-major loads only after the first score operand
            # is in flight: the SDMA engines round-robin both HWDGE rings,
            # so an early xn dispatch would starve the matmul-gating xtp.
            scalar.wait_ge(sQ1, 16)
            scalar.dma_start(out=xn_sb[:, :HS], in_=xn[:, :HS]).then_inc(aQ0, 16)
            scalar.dma_start(out=xn_sb[:, HS:], in_=xn[:, HS:]).then_inc(aQ1, 16)
            scalar.dma_start(out=codes_sb[:, :], in_=codes_in[:, :]).then_inc(aQ2, 16)
            scalar.wait_ge(aQ0, 16)
            scalar.activation(out=scr[0][:, :], in_=fc(xn_sb[:, :HS]), func=AFT.Square).then_inc(sqS)
            scalar.wait_ge(aQ1, 16)
            scalar.activation(out=scr[1][:, :], in_=fc(xn_sb[:, HS:]), func=AFT.Square).then_inc(sqS)
            scalar.wait_ge(dvA, 1)
            scalar.activation(out=l_t[0][:, :], in_=a_t[0][:, :], func=AFT.Ln).then_inc(lnN)
            scalar.wait_ge(dvT, 1)
            scalar.activation(out=p_t[0][:, :], in_=t_t[0][:, :], func=AFT.Exp, scale=0.5)
            scalar.drain()
            scalar.activation(out=ew_t[0][:, :], in_=p_t[0][:, :], func=AFT.Exp).then_inc(ewN)
            scalar.wait_ge(dvA, 2)
            scalar.activation(out=l_t[1][:, :], in_=a_t[1][:, :], func=AFT.Ln).then_inc(lnN)
            scalar.wait_ge(dvT, 2)
            scalar.activation(out=p_t[1][:, :], in_=t_t[1][:, :], func=AFT.Exp, scale=0.5)
            scalar.drain()
            scalar.activation(out=ew_t[1][:, :], in_=p_t[1][:, :], func=AFT.Exp).then_inc(ewN)

        @block.tensor
        def _(tensor):
            tensor.wait_ge(sQr, 16)
            tensor.wait_ge(sQ1, 16)
            for i in range(TPS // 2):
                nc.tensor.matmul(
                    ps[0][:, i * K : (i + 1) * K],
                    xtp_sb[:, i * 128 : (i + 1) * 128],
                    ct2v, start=True, stop=True,
                )
            tensor.wait_ge(sQ2, 16)
            for i in range(TPS // 2, TPS):
                nc.tensor.matmul(
                    ps[0][:, i * K : (i + 1) * K],
                    xtp_sb[:, i * 128 : (i + 1) * 128],
                    ct2v, start=True, stop=True,
                ).then_inc(mmS) if i == TPS - 1 else nc.tensor.matmul(
                    ps[0][:, i * K : (i + 1) * K],
                    xtp_sb[:, i * 128 : (i + 1) * 128],
                    ct2v, start=True, stop=True,
                )
            tensor.wait_ge(sQ3, 16)
            for i in range(TPS):
                mm = nc.tensor.matmul(
                    ps[1][:, i * K : (i + 1) * K],
                    xtp_sb[:, HT + i * 128 : HT + (i + 1) * 128],
                    ct2v, start=True, stop=True,
                )
                if i == TPS - 1:
                    mm.then_inc(mmS)
            tensor.wait_ge(wR, 1)
            for i in range(TPS):
                nc.tensor.matmul(
                    pms[:, :],
                    w_t[0][:, i * K : (i + 1) * K],
                    xn_sb[:, i * XNW : (i + 1) * XNW],
                    start=(i == 0), stop=False,
                )
            tensor.wait_ge(wR, 2)
            for i in range(TPS):
                mm = nc.tensor.matmul(
                    pms[:, :],
                    w_t[1][:, i * K : (i + 1) * K],
                    xn_sb[:, HS + i * XNW : HS + (i + 1) * XNW],
                    start=False, stop=(i == TPS - 1),
                )
                if i == TPS - 1:
                    mm.then_inc(aggS)

        @block.vector
        def _(vector):
            AX = mybir.AxisListType.X
            vector.wait_ge(sQc, 16)
            vector.wait_ge(sqS, 1)
            nc.vector.tensor_reduce(
                out=xxp[0][:, :], in_=t3(scr[0][:, :], k=XNW), axis=AX, op=ALU.add
            )
            vector.drain()
            nc.vector.tensor_add(t3(ccxx[0][:, :]), bct(ccbv, TPS), bck(xxp[0][:, :], K))
            vector.drain()
            vector.wait_ge(mmS, 1)
            nc.vector.tensor_add(a_t[0][:, :], ps[0][:, : TPS * K], ccxx[0][:, :]).then_inc(dvA)
            vector.wait_ge(lnN, 1)
            nc.vector.tensor_add(
                t3(t_t[0][:, :]), t3(l_t[0][:, :]), bct(lnsv, TPS)
            ).then_inc(dvT)
            vector.wait_ge(sqS, 2)
            nc.vector.tensor_reduce(
                out=xxp[1][:, :], in_=t3(scr[1][:, :], k=XNW), axis=AX, op=ALU.add
            )
            vector.drain()
            nc.vector.tensor_add(t3(ccxx[1][:, :]), bct(ccbv, TPS), bck(xxp[1][:, :], K))
            vector.drain()
            vector.wait_ge(mmS, 2)
            nc.vector.tensor_add(a_t[1][:, :], ps[1][:, : TPS * K], ccxx[1][:, :]).then_inc(dvA)
            vector.wait_ge(ewN, 1)
            nc.vector.tensor_reduce(
                out=den[0][:, :], in_=t3(ew_t[0][:, :]), axis=AX, op=ALU.add
            )
            vector.drain()
            nc.vector.reciprocal(out=rden[0][:, :], in_=den[0][:, :])
            vector.drain()
            nc.vector.tensor_mul(
                t3(w_t[0][:, :]), t3(ew_t[0][:, :]), bck(rden[0][:, :], K)
            ).then_inc(wR)
            vector.wait_ge(lnN, 2)
            nc.vector.tensor_add(
                t3(t_t[1][:, :]), t3(l_t[1][:, :]), bct(lnsv, TPS)
            ).then_inc(dvT)
            vector.wait_ge(ewN, 2)
            nc.vector.tensor_reduce(
                out=den[1][:, :], in_=t3(ew_t[1][:, :]), axis=AX, op=ALU.add
            )
            vector.drain()
            nc.vector.reciprocal(out=rden[1][:, :], in_=den[1][:, :])
            vector.drain()
            nc.vector.tensor_mul(
                t3(w_t[1][:, :]), t3(ew_t[1][:, :]), bck(rden[1][:, :], K)
            ).then_inc(wR)
            vector.wait_ge(aggS, 1)
            vector.wait_ge(aQ2, 16)
            nc.vector.tensor_copy(s_sb[:, :], pms[:, 128:129])
            vector.drain()
            nc.vector.tensor_scalar_mul(tmp_sb[:, :], codes_sb[:, :], s_sb[:, :])
            vector.drain()
            nc.vector.tensor_sub(e_sb[:, :], pms[:, 0:128], tmp_sb[:, :]).then_inc(eR)

    nc.compile()
    return nc

def _get_nc():
    if "nc" not in _CACHE:
        _CACHE["nc"] = _build_nc_raw() if _RAW else _build_nc()
    return _CACHE["nc"]


def _prep_inputs(x, codes, scale):
    """Build the per-core input maps (all host-side numpy)."""
    x = np.asarray(x, dtype=np.float32).reshape(N, D)
    codes = np.asarray(codes, dtype=np.float32)
    scale = np.asarray(scale, dtype=np.float32)

    ct2 = np.ascontiguousarray(-2.0 * codes.T)                      # [D, K]
    cc = (codes * codes).sum(axis=1).astype(np.float32)             # [K]
    lns2 = 2.0 * np.log(np.maximum(scale, np.float32(1e-30)))
    cst = np.ascontiguousarray(
        np.broadcast_to(
            np.concatenate([cc - 1.0, lns2]).astype(np.float32)[None, :],
            (128, 2 * K),
        )
    )

    in_maps = []
    for core in range(N_CORES):
        xs = x[core * NPC : (core + 1) * NPC]                       # [2048, 128]
        a = xs.reshape(128, NT, D)                                  # [p, t, d]
        xnv = np.concatenate(
            [
                a,
                np.ones((128, NT, 1), dtype=np.float32),
                np.zeros((128, NT, 1), dtype=np.float32),
            ],
            axis=2,
        ).reshape(128, NT * XNW)
        xtpv = np.ascontiguousarray(a.transpose(2, 1, 0)).reshape(128, NPC)
        m = {
            "xn": np.ascontiguousarray(xnv),
            "xtp": xtpv,
            "cst": cst,
            "codes": codes,
        }
        if _RAW:
            m["cstr"] = ct2
        if not _RAW:
            m["ct2"] = ct2
            m["cst"] = np.ascontiguousarray(
                np.broadcast_to(
                    np.concatenate([cc - 1.0, scale, lns2]).astype(np.float32)[
                        None, :
                    ],
                    (128, 3 * K),
                )
            )
        in_maps.append(m)
    return in_maps


def kernel(x, codes, scale):
    from concourse.bass_utils import run_bass_kernel_spmd

    nc = _get_nc()
    in_maps = _prep_inputs(x, codes, scale)
    res = run_bass_kernel_spmd(nc, in_maps, core_ids=list(range(N_CORES)))
    out = np.zeros((K, D), dtype=np.float32)
    for r in res.results:
        out += r["E"]
    return out


# revision 20
# speedup vs baseline: 1.1048x; 1.1048x over previous
"""VQ codebook encoding kernel for Trainium2, sharded over 8 NeuronCores.

Math (per shard of N tokens):
    l2[n,k]  = ||x_n - c_k||            (NOT squared)
    W        = softmax_k(l2 * scale_k)
    E[k,d]   = sum_n W[n,k] * (x[n,d] - c[k,d])
             = (W^T X)[k,d] - S_k * c[k,d],   S_k = sum_n W[n,k]

so we never materialize the (N,K,D) residual tensor.  The N axis is
sharded 8 ways; each core returns a partial (K,D) E which the host sums.

Layout tricks:
- x is fed twice per core: natural [n,D] rows (aggregation matmul; a
  [1,0] pad per 128-token tile makes one matmul yield W^T[X|1|0] =
  [M | S | 0] and keeps free dims even for fp32r) and host-pre-transposed
  [D,n] (score matmuls).  Host transposes are free and kill all on-chip
  transposes.
- ||x_n||^2: square the [x|1|0] tile wholesale (ACT) + segmented reduce
  (DVE); the ones column contributes +1, folded into the (cc-1) const.
- softmax: logits = l2*s = exp(0.5*(ln(l2^2) + ln(s^2))); hw_specs'
  cached activation-table map is seeded so ln/exp/square all resolve to
  the one table set containing all three -> exactly one ACT_TABLE_LOAD.
- matmuls run float32r (single-pass on the PE) -- fp32 is 2-pass.
- per-k constants (cc-1, s, ln s^2) are fed as [128,32] tiles and read
  with stride-0 APs to broadcast across the 16 token-tiles; DVE free-dim
  step-0 broadcasts also splice ||x||^2 and 1/den per token-tile.
- DMA order is chosen so the score-matmul operands land first (HWDGE
  transfers complete in FIFO order).
"""

import sys

if "/opt/trn_rl_repo" not in sys.path:
    sys.path.insert(0, "/opt/trn_rl_repo")

import os as _os

import numpy as np

N_CORES = 8
N, K, D = 16384, 32, 128
NPC = N // N_CORES          # tokens per core = 2048
NT = NPC // 128             # 128-token tiles per core = 16
SC = 2                      # superchunks
TPS = NT // SC              # tiles per superchunk = 8
XNW = 130                   # x tile width incl. ones col + pad (fp32r wants even)

_CACHE = {}

_MMDT = _os.environ.get("KMMDT", "f32r")            # f32r | f32
_RAW = _os.environ.get("KRAW", "1") == "1"          # raw bass vs TileContext


def _force_combined_act_table(nc, mybir):
    """Seed hw_specs' cached activation-table dict so the ln/exp/square
    activations all resolve to the one set that contains all three
    ("natural_log_exp_and_others"), giving a single ACT_TABLE_LOAD instead
    of per-func table thrash.  Only mutates the per-process cache copy;
    set ids stay aligned with the compiler's act_info.json."""
    import concourse.hw_specs as hw_specs

    AFT = mybir.ActivationFunctionType
    tables = hw_specs.get_activation_tables(nc.m.arch)
    if "natural_log_exp_and_others" not in tables:
        return
    for name, funcs in tables.items():
        if name != "natural_log_exp_and_others":
            funcs.discard(AFT.Exp)
            funcs.discard(AFT.Ln)
            funcs.discard(AFT.Square)


def _build_nc():
    import concourse.bacc as bacc
    import concourse.bass as bass
    import concourse.mybir as mybir
    from concourse.tile import TileContext

    f32 = mybir.dt.float32
    f32r = mybir.dt.float32r
    AFT = mybir.ActivationFunctionType
    ALU = mybir.AluOpType

    mmdt = f32r if _MMDT == "f32r" else f32

    nc = bacc.Bacc(None, target_bir_lowering=False)
    _force_combined_act_table(nc, mybir)

    xn = nc.dram_tensor("xn", [128, NT * XNW], mmdt, kind="ExternalInput")
    xtp = nc.dram_tensor("xtp", [128, NPC], mmdt, kind="ExternalInput")
    ct2 = nc.dram_tensor("ct2", [128, K], mmdt, kind="ExternalInput")
    # packed per-k consts: [cc-1 | s | ln s^2], each [128, K]
    cst = nc.dram_tensor("cst", [128, 3 * K], f32, kind="ExternalInput")
    codes_in = nc.dram_tensor("codes", [K, D], f32, kind="ExternalInput")
    e_out = nc.dram_tensor("E", [K, D], f32, kind="ExternalOutput")

    def fcast(ap):
        # view an mmdt tile as plain f32 for non-matmul consumers
        return ap.bitcast(f32) if mmdt is f32r else ap

    def bcast_t(ap32, count):
        # [128, 32] const -> [128, count, 32] via a stride-0 middle dim
        return bass.AP(
            tensor=ap32.tensor,
            offset=ap32.offset,
            ap=[list(ap32.ap[0]), [0, count], list(ap32.ap[1])],
        )

    def bcast_k(apw, count):
        # [128, w] per-tile scalars -> [128, w, count] via stride-0 inner dim
        return bass.AP(
            tensor=apw.tensor,
            offset=apw.offset,
            ap=[list(apw.ap[0]), list(apw.ap[1]), [0, count]],
        )

    with TileContext(nc) as tc:
        with (
            tc.tile_pool(name="singles", bufs=1) as singles,
            tc.tile_pool(name="data", bufs=SC) as data,
            tc.tile_pool(name="work", bufs=SC) as work,
            tc.tile_pool(name="psum_sc", bufs=SC, space="PSUM") as psum_sc,
            tc.tile_pool(name="psum_ag", bufs=1, space="PSUM") as psum_ag,
        ):
            # ---- tiny consts first (they gate everything) ----
            ct2_sb = singles.tile([128, K], mmdt)
            nc.sync.dma_start(out=ct2_sb, in_=ct2[:, :])
            cst_sb = singles.tile([128, 3 * K], f32)
            nc.sync.dma_start(out=cst_sb, in_=cst[:, :])
            ccb_sb = cst_sb[:, 0:K]
            sclb_sb = cst_sb[:, K : 2 * K]
            lns2_sb = cst_sb[:, 2 * K : 3 * K]

            # ---- data DMAs, in the order compute needs them ----
            xn_chunks = []
            xtp_chunks = []
            for c in range(SC):
                xn_c = data.tile([128, TPS * XNW], mmdt, tag="xn")
                xtp_c = data.tile([128, TPS * 128], mmdt, tag="xtp")
                xn_chunks.append(xn_c)
                xtp_chunks.append(xtp_c)
            # first half of xtp0 lands first so matmuls can start asap
            h = TPS * 128 // 2
            nc.sync.dma_start(out=xtp_chunks[0][:, :h], in_=xtp[:, :h])
            nc.sync.dma_start(out=xtp_chunks[0][:, h:], in_=xtp[:, h : TPS * 128])
            nc.sync.dma_start(out=xn_chunks[0], in_=xn[:, : TPS * XNW])
            nc.sync.dma_start(out=xtp_chunks[1], in_=xtp[:, TPS * 128 : 2 * TPS * 128])
            nc.sync.dma_start(out=xn_chunks[1], in_=xn[:, TPS * XNW : 2 * TPS * XNW])
            codes_sb = singles.tile([K, D], f32)
            nc.sync.dma_start(out=codes_sb, in_=codes_in[:, :])

            psum_ms = psum_ag.tile([K, XNW], f32)
            w_chunks = []

            # ---- phase A: scores + softmax weights, per superchunk ----
            for c in range(SC):
                xn_c = xn_chunks[c]
                xtp_c = xtp_chunks[c]

                # scores: -2 x.c for 8 tiles into one PSUM bank
                ps_c = psum_sc.tile([128, TPS * K], f32)
                for i in range(TPS):
                    nc.tensor.matmul(
                        ps_c[:, i * K : (i + 1) * K],
                        xtp_c[:, i * 128 : (i + 1) * 128],
                        ct2_sb,
                        start=True,
                        stop=True,
                    )

                # xx+1 per (token, tile): square the whole [x|1|0] superchunk
                scr_c = work.tile([128, TPS * XNW], f32, tag="scr")
                nc.scalar.activation(out=scr_c, in_=fcast(xn_c), func=AFT.Square)
                xxp_c = work.tile([128, TPS], f32, tag="xxp")
                nc.vector.tensor_reduce(
                    out=xxp_c,
                    in_=scr_c.rearrange("p (t w) -> p t w", w=XNW),
                    axis=mybir.AxisListType.X,
                    op=ALU.add,
                )

                # ccxx[p,t,k] = (cc_k-1) + (xx+1)[p,t]  (independent of PE)
                ccxx_c = work.tile([128, TPS * K], f32, tag="ccxx")
                ccxx3 = ccxx_c.rearrange("p (t k) -> p t k", k=K)
                nc.vector.tensor_add(ccxx3, bcast_t(ccb_sb, TPS), bcast_k(xxp_c, K))

                # A = l2^2 = -2xc + ccxx
                a_c = work.tile([128, TPS * K], f32, tag="a")
                nc.vector.tensor_add(a_c, ps_c, ccxx_c)

                # logits = l2*s = exp(0.5*(ln(l2^2) + ln(s^2)))
                l_c = work.tile([128, TPS * K], f32, tag="l")
                nc.scalar.activation(out=l_c, in_=a_c, func=AFT.Ln)
                t_c = work.tile([128, TPS * K], f32, tag="t")
                nc.vector.tensor_add(
                    t_c.rearrange("p (t k) -> p t k", k=K),
                    l_c.rearrange("p (t k) -> p t k", k=K),
                    bcast_t(lns2_sb, TPS),
                )
                p_c = work.tile([128, TPS * K], f32, tag="p")
                nc.scalar.activation(out=p_c, in_=t_c, func=AFT.Exp, scale=0.5)
                # EW = exp(logits)
                ew_c = work.tile([128, TPS * K], f32, tag="ew")
                nc.scalar.activation(out=ew_c, in_=p_c, func=AFT.Exp)

                # denominators + reciprocal + normalize
                den_c = work.tile([128, TPS], f32, tag="den")
                nc.vector.tensor_reduce(
                    out=den_c,
                    in_=ew_c.rearrange("p (t k) -> p t k", k=K),
                    axis=mybir.AxisListType.X,
                    op=ALU.add,
                )
                rden_c = work.tile([128, TPS], f32, tag="rden")
                nc.vector.reciprocal(out=rden_c, in_=den_c)

                w_c = work.tile([128, TPS * K], mmdt, tag="w")
                nc.vector.tensor_mul(
                    w_c.rearrange("p (t k) -> p t k", k=K),
                    ew_c.rearrange("p (t k) -> p t k", k=K),
                    bcast_k(rden_c, K),
                )
                w_chunks.append(w_c)

            # ---- phase B: aggregation matmuls ----
            for c in range(SC):
                for i in range(TPS):
                    t = c * TPS + i
                    nc.tensor.matmul(
                        psum_ms,
                        w_chunks[c][:, i * K : (i + 1) * K],
                        xn_chunks[c][:, i * XNW : (i + 1) * XNW],
                        start=(t == 0),
                        stop=(t == NT - 1),
                    )

            # ---- final: E = M - S * codes ----
            s_sb = singles.tile([K, 1], f32)
            nc.vector.tensor_copy(s_sb, psum_ms[:, 128:129])
            tmp = singles.tile([K, D], f32)
            nc.vector.tensor_scalar_mul(tmp, codes_sb, s_sb)
            e_sb = singles.tile([K, D], f32)
            nc.vector.tensor_sub(e_sb, psum_ms[:, 0:128], tmp)
            nc.sync.dma_start(out=e_out[:, :], in_=e_sb)

    nc.compile()
    return nc




def _build_nc_raw():
    """Raw-bass variant: same pipeline as the Tile builder but with
    hand-rolled semaphores and no TileContext, which drops the ~10us
    end-of-kernel drain+barrier teardown and most per-op sync overhead.
    DMA dispatches are split across the two HWDGE queues (sync + scalar)."""
    import concourse.bacc as bacc
    import concourse.bass as bass
    import concourse.mybir as mybir

    f32 = mybir.dt.float32
    f32r = mybir.dt.float32r
    AFT = mybir.ActivationFunctionType
    ALU = mybir.AluOpType

    mmdt = f32r if _MMDT == "f32r" else f32

    nc = bacc.Bacc(None, target_bir_lowering=False)
    _force_combined_act_table(nc, mybir)

    HS = TPS * XNW                 # xn elems per superchunk (1040)
    HT = TPS * 128                 # xtp elems per superchunk (1024)

    xn = nc.dram_tensor("xn", [128, NT * XNW], mmdt, kind="ExternalInput")
    xtp = nc.dram_tensor("xtp", [128, NPC], mmdt, kind="ExternalInput")
    cstr = nc.dram_tensor("cstr", [128, K], mmdt, kind="ExternalInput")   # ct2
    cst = nc.dram_tensor("cst", [128, 2 * K], f32, kind="ExternalInput")  # cc-1|lns2
    codes_in = nc.dram_tensor("codes", [K, D], f32, kind="ExternalInput")
    e_out = nc.dram_tensor("E", [K, D], f32, kind="ExternalOutput")

    sb = lambda name, shape, dt: nc.alloc_sbuf_tensor(name, shape, dt)
    ct2_sb = sb("ct2_sb", [128, K], mmdt)
    cst_sb = sb("cst_sb", [128, 2 * K], f32)
    xtp_sb = sb("xtp_sb", [128, NPC], mmdt)
    xn_sb = sb("xn_sb", [128, NT * XNW], mmdt)
    codes_sb = sb("codes_sb", [K, D], f32)
    scr = [sb(f"scr{c}", [128, HS], f32) for c in range(SC)]
    xxp = [sb(f"xxp{c}", [128, TPS], f32) for c in range(SC)]
    ccxx = [sb(f"ccxx{c}", [128, TPS * K], f32) for c in range(SC)]
    a_t = [sb(f"a{c}", [128, TPS * K], f32) for c in range(SC)]
    l_t = [sb(f"l{c}", [128, TPS * K], f32) for c in range(SC)]
    t_t = [sb(f"t{c}", [128, TPS * K], f32) for c in range(SC)]
    p_t = [sb(f"p{c}", [128, TPS * K], f32) for c in range(SC)]
    ew_t = [sb(f"ew{c}", [128, TPS * K], f32) for c in range(SC)]
    den = [sb(f"den{c}", [128, TPS], f32) for c in range(SC)]
    rden = [sb(f"rden{c}", [128, TPS], f32) for c in range(SC)]
    w_t = [sb(f"w{c}", [128, TPS * K], mmdt) for c in range(SC)]
    s_sb = sb("s_sb", [K, 1], f32)
    tmp_sb = sb("tmp_sb", [K, D], f32)
    e_sb = sb("e_sb", [K, D], f32)

    # full-bank allocations so PE writes and DVE reads never share a bank
    ps = [nc.alloc_psum_tensor(f"ps{c}", [128, 512], f32) for c in range(SC)]
    pms = nc.alloc_psum_tensor("pms", [K, XNW], f32)

    ct2v = ct2_sb[:, :]
    ccbv = cst_sb[:, 0:K]
    lnsv = cst_sb[:, K : 2 * K]

    def fc(ap):
        return ap.bitcast(f32) if mmdt is f32r else ap

    def bct(ap32, count):
        return bass.AP(
            tensor=ap32.tensor,
            offset=ap32.offset,
            ap=[list(ap32.ap[0]), [0, count], list(ap32.ap[1])],
        )

    def bck(apw, count):
        return bass.AP(
            tensor=apw.tensor,
            offset=apw.offset,
            ap=[list(apw.ap[0]), list(apw.ap[1]), [0, count]],
        )

    def t3(ap, k=K):
        return ap.rearrange("p (t k) -> p t k", k=k)

    sQr = nc.alloc_semaphore("sQr")      # ct2 DMA done
    sQc = nc.alloc_semaphore("sQc")      # cst DMA done
    sQ1 = nc.alloc_semaphore("sQ1")      # xtp first half
    sQ2 = nc.alloc_semaphore("sQ2")      # xtp second half
    sQ3 = nc.alloc_semaphore("sQ3")      # xtp sc1
    aQ0 = nc.alloc_semaphore("aQ0")      # xn sc0
    aQ1 = nc.alloc_semaphore("aQ1")      # xn sc1
    aQ2 = nc.alloc_semaphore("aQ2")      # codes
    mmS = nc.alloc_semaphore("mmS")      # PE: score matmuls done per sc
    aggS = nc.alloc_semaphore("aggS")    # PE: aggregation done
    sqS = nc.alloc_semaphore("sqS")      # ACT squares done per sc
    lnN = nc.alloc_semaphore("lnN")      # ACT ln done per sc
    ewN = nc.alloc_semaphore("ewN")      # ACT exp(exp) done per sc
    dvA = nc.alloc_semaphore("dvA")      # DVE A=l2^2 ready per sc
    dvT = nc.alloc_semaphore("dvT")      # DVE t=ln(l2^2 s^2) ready per sc
    wR = nc.alloc_semaphore("wR")        # DVE W ready per sc
    eR = nc.alloc_semaphore("eR")        # E ready in SBUF
    oD = nc.alloc_semaphore("oD")        # output DMA done

    with nc.Block(no_gpsimd_drain=True) as block:

        @block.sync
        def _(sync):
            sync.dma_start(out=ct2_sb[:, :], in_=cstr[:, :]).then_inc(sQr, 16)
            sync.dma_start(out=cst_sb[:, :], in_=cst[:, :]).then_inc(sQc, 16)
            h = HT // 2
            sync.dma_start(out=xtp_sb[:, :h], in_=xtp[:, :h]).then_inc(sQ1, 16)
            sync.dma_start(out=xtp_sb[:, h:HT], in_=xtp[:, h:HT]).then_inc(sQ2, 16)
            sync.dma_start(out=xtp_sb[:, HT:], in_=xtp[:, HT:]).then_inc(sQ3, 16)
            sync.wait_ge(eR, 1)
            sync.dma_start(out=e_out[:, :], in_=e_sb[:, :]).then_inc(oD, 16)
            sync.wait_ge(oD, 16)

        @block.scalar
        def _(scalar):
            scalar.dma_start(out=xn_sb[:, :HS], in_=xn[:, :HS]).then_inc(aQ0, 16)
            scalar.dma_start(out=xn_sb[:, HS:], in_=xn[:, HS:]).then_inc(aQ1, 16)
            scalar.dma_start(out=codes_sb[:, :], in_=codes_in[:, :]).then_inc(aQ2, 16)
            scalar.wait_ge(aQ0, 16)
            scalar.activation(out=scr[0][:, :], in_=fc(xn_sb[:, :HS]), func=AFT.Square).then_inc(sqS)
            scalar.wait_ge(aQ1, 16)
            scalar.activation(out=scr[1][:, :], in_=fc(xn_sb[:, HS:]), func=AFT.Square).then_inc(sqS)
            scalar.wait_ge(dvA, 1)
            scalar.activation(out=l_t[0][:, :], in_=a_t[0][:, :], func=AFT.Ln).then_inc(lnN)
            scalar.wait_ge(dvT, 1)
            scalar.activation(out=p_t[0][:, :], in_=t_t[0][:, :], func=AFT.Exp, scale=0.5)
            scalar.drain()
            scalar.activation(out=ew_t[0][:, :], in_=p_t[0][:, :], func=AFT.Exp).then_inc(ewN)
            scalar.wait_ge(dvA, 2)
            scalar.activation(out=l_t[1][:, :], in_=a_t[1][:, :], func=AFT.Ln).then_inc(lnN)
            scalar.wait_ge(dvT, 2)
            scalar.activation(out=p_t[1][:, :], in_=t_t[1][:, :], func=AFT.Exp, scale=0.5)
            scalar.drain()
            scalar.activation(out=ew_t[1][:, :], in_=p_t[1][:, :], func=AFT.Exp).then_inc(ewN)

        @block.tensor
        def _(tensor):
            tensor.wait_ge(sQr, 16)
            tensor.wait_ge(sQ1, 16)
            for i in range(TPS // 2):
                nc.tensor.matmul(
                    ps[0][:, i * K : (i + 1) * K],
                    xtp_sb[:, i * 128 : (i + 1) * 128],
                    ct2v, start=True, stop=True,
                )
            tensor.wait_ge(sQ2, 16)
            for i in range(TPS // 2, TPS):
                nc.tensor.matmul(
                    ps[0][:, i * K : (i + 1) * K],
                    xtp_sb[:, i * 128 : (i + 1) * 128],
                    ct2v, start=True, stop=True,
                ).then_inc(mmS) if i == TPS - 1 else nc.tensor.matmul(
                    ps[0][:, i * K : (i + 1) * K],
                    xtp_sb[:, i * 128 : (i + 1) * 128],
                    ct2v, start=True, stop=True,
                )
            tensor.wait_ge(sQ3, 16)
            for i in range(TPS):
                mm = nc.tensor.matmul(
                    ps[1][:, i * K : (i + 1) * K],
                    xtp_sb[:, HT + i * 128 : HT + (i + 1) * 128],
                    ct2v, start=True, stop=True,
                )
                if i == TPS - 1:
                    mm.then_inc(mmS)
            tensor.wait_ge(wR, 1)
            for i in range(TPS):
                nc.tensor.matmul(
                    pms[:, :],
                    w_t[0][:, i * K : (i + 1) * K],
                    xn_sb[:, i * XNW : (i + 1) * XNW],
                    start=(i == 0), stop=False,
                )
            tensor.wait_ge(wR, 2)
            for i in range(TPS):
                mm = nc.tensor.matmul(
                    pms[:, :],
                    w_t[1][:, i * K : (i + 1) * K],
                    xn_sb[:, HS + i * XNW : HS + (i + 1) * XNW],
                    start=False, stop=(i == TPS - 1),
                )
                if i == TPS - 1:
                    mm.then_inc(aggS)

        @block.vector
        def _(vector):
            AX = mybir.AxisListType.X
            vector.wait_ge(sQc, 16)
            vector.wait_ge(sqS, 1)
            nc.vector.tensor_reduce(
                out=xxp[0][:, :], in_=t3(scr[0][:, :], k=XNW), axis=AX, op=ALU.add
            )
            vector.drain()
            nc.vector.tensor_add(t3(ccxx[0][:, :]), bct(ccbv, TPS), bck(xxp[0][:, :], K))
            vector.drain()
            vector.wait_ge(mmS, 1)
            nc.vector.tensor_add(a_t[0][:, :], ps[0][:, : TPS * K], ccxx[0][:, :]).then_inc(dvA)
            vector.wait_ge(lnN, 1)
            nc.vector.tensor_add(
                t3(t_t[0][:, :]), t3(l_t[0][:, :]), bct(lnsv, TPS)
            ).then_inc(dvT)
            vector.wait_ge(sqS, 2)
            nc.vector.tensor_reduce(
                out=xxp[1][:, :], in_=t3(scr[1][:, :], k=XNW), axis=AX, op=ALU.add
            )
            vector.drain()
            nc.vector.tensor_add(t3(ccxx[1][:, :]), bct(ccbv, TPS), bck(xxp[1][:, :], K))
            vector.drain()
            vector.wait_ge(mmS, 2)
            nc.vector.tensor_add(a_t[1][:, :], ps[1][:, : TPS * K], ccxx[1][:, :]).then_inc(dvA)
            vector.wait_ge(ewN, 1)
            nc.vector.tensor_reduce(
                out=den[0][:, :], in_=t3(ew_t[0][:, :]), axis=AX, op=ALU.add
            )
            vector.drain()
            nc.vector.reciprocal(out=rden[0][:, :], in_=den[0][:, :])
            vector.drain()
            nc.vector.tensor_mul(
                t3(w_t[0][:, :]), t3(ew_t[0][:, :]), bck(rden[0][:, :], K)
            ).then_inc(wR)
            vector.wait_ge(lnN, 2)
            nc.vector.tensor_add(
                t3(t_t[1][:, :]), t3(l_t[1][:, :]), bct(lnsv, TPS)
            ).then_inc(dvT)
            vector.wait_ge(ewN, 2)
            nc.vector.tensor_reduce(
                out=den[1][:, :], in_=t3(ew_t[1][:, :]), axis=AX, op=ALU.add
            )
            vector.drain()
            nc.vector.reciprocal(out=rden[1][:, :], in_=den[1][:, :])
            vector.drain()
            nc.vector.tensor_mul(
                t3(w_t[1][:, :]), t3(ew_t[1][:, :]), bck(rden[1][:, :], K)
            ).then_inc(wR)
            vector.wait_ge(aggS, 1)
            vector.wait_ge(aQ2, 16)
            nc.vector.tensor_copy(s_sb[:, :], pms[:, 128:129])
            vector.drain()
            nc.vector.tensor_scalar_mul(tmp_sb[:, :], codes_sb[:, :], s_sb[:, :])
            vector.drain()
            nc.vector.tensor_sub(e_sb[:, :], pms[:, 0:128], tmp_sb[:, :]).then_inc(eR)

    nc.compile()
    return nc

def _get_nc():
    if "nc" not in _CACHE:
        _CACHE["nc"] = _build_nc_raw() if _RAW else _build_nc()
    return _CACHE["nc"]


def _prep_inputs(x, codes, scale):
    """Build the per-core input maps (all host-side numpy)."""
    x = np.asarray(x, dtype=np.float32).reshape(N, D)
    codes = np.asarray(codes, dtype=np.float32)
    scale = np.asarray(scale, dtype=np.float32)

    ct2 = np.ascontiguousarray(-2.0 * codes.T)                      # [D, K]
    cc = (codes * codes).sum(axis=1).astype(np.float32)             # [K]
    lns2 = 2.0 * np.log(np.maximum(scale, np.float32(1e-30)))
    cst = np.ascontiguousarray(
        np.broadcast_to(
            np.concatenate([cc - 1.0, lns2]).astype(np.float32)[None, :],
            (128, 2 * K),
        )
    )

    in_maps = []
    for core in range(N_CORES):
        xs = x[core * NPC : (core + 1) * NPC]                       # [2048, 128]
        a = xs.reshape(128, NT, D)                                  # [p, t, d]
        xnv = np.concatenate(
            [
                a,
                np.ones((128, NT, 1), dtype=np.float32),
                np.zeros((128, NT, 1), dtype=np.float32),
            ],
            axis=2,
        ).reshape(128, NT * XNW)
        xtpv = np.ascontiguousarray(a.transpose(2, 1, 0)).reshape(128, NPC)
        m = {
            "xn": np.ascontiguousarray(xnv),
            "xtp": xtpv,
            "cst": cst,
            "codes": codes,
        }
        if _RAW:
            m["cstr"] = ct2
        if not _RAW:
            m["ct2"] = ct2
            m["cst"] = np.ascontiguousarray(
                np.broadcast_to(
                    np.concatenate([cc - 1.0, scale, lns2]).astype(np.float32)[
                        None, :
                    ],
                    (128, 3 * K),
                )
            )
        in_maps.append(m)
    return in_maps


def kernel(x, codes, scale):
    from concourse.bass_utils import run_bass_kernel_spmd

    nc = _get_nc()
    in_maps = _prep_inputs(x, codes, scale)
    res = run_bass_kernel_spmd(nc, in_maps, core_ids=list(range(N_CORES)))
    out = np.zeros((K, D), dtype=np.float32)
    for r in res.results:
        out += r["E"]
    return out


# revision 21
# speedup vs baseline: 1.1073x; 1.0022x over previous
"""VQ codebook encoding kernel for Trainium2, sharded over 8 NeuronCores.

Math (per shard of N tokens):
    l2[n,k]  = ||x_n - c_k||            (NOT squared)
    W        = softmax_k(l2 * scale_k)
    E[k,d]   = sum_n W[n,k] * (x[n,d] - c[k,d])
             = (W^T X)[k,d] - S_k * c[k,d],   S_k = sum_n W[n,k]

so we never materialize the (N,K,D) residual tensor.  The N axis is
sharded 8 ways; each core returns a partial (K,D) E which the host sums.

Layout tricks:
- x is fed twice per core: natural [n,D] rows (aggregation matmul; a
  [1,0] pad per 128-token tile makes one matmul yield W^T[X|1|0] =
  [M | S | 0] and keeps free dims even for fp32r) and host-pre-transposed
  [D,n] (score matmuls).  Host transposes are free and kill all on-chip
  transposes.
- ||x_n||^2: square the [x|1|0] tile wholesale (ACT) + segmented reduce
  (DVE); the ones column contributes +1, folded into the (cc-1) const.
- softmax: logits = l2*s = exp(0.5*(ln(l2^2) + ln(s^2))); hw_specs'
  cached activation-table map is seeded so ln/exp/square all resolve to
  the one table set containing all three -> exactly one ACT_TABLE_LOAD.
- matmuls run float32r (single-pass on the PE) -- fp32 is 2-pass.
- per-k constants (cc-1, s, ln s^2) are fed as [128,32] tiles and read
  with stride-0 APs to broadcast across the 16 token-tiles; DVE free-dim
  step-0 broadcasts also splice ||x||^2 and 1/den per token-tile.
- DMA order is chosen so the score-matmul operands land first (HWDGE
  transfers complete in FIFO order).
"""

import sys

if "/opt/trn_rl_repo" not in sys.path:
    sys.path.insert(0, "/opt/trn_rl_repo")

import os as _os

import numpy as np

N_CORES = 8
N, K, D = 16384, 32, 128
NPC = N // N_CORES          # tokens per core = 2048
NT = NPC // 128             # 128-token tiles per core = 16
SC = 2                      # superchunks
TPS = NT // SC              # tiles per superchunk = 8
XNW = 130                   # x tile width incl. ones col + pad (fp32r wants even)

_CACHE = {}

_MMDT = _os.environ.get("KMMDT", "f32r")            # f32r | f32
_RAW = _os.environ.get("KRAW", "1") == "1"          # raw bass vs TileContext


def _force_combined_act_table(nc, mybir):
    """Seed hw_specs' cached activation-table dict so the ln/exp/square
    activations all resolve to the one set that contains all three
    ("natural_log_exp_and_others"), giving a single ACT_TABLE_LOAD instead
    of per-func table thrash.  Only mutates the per-process cache copy;
    set ids stay aligned with the compiler's act_info.json."""
    import concourse.hw_specs as hw_specs

    AFT = mybir.ActivationFunctionType
    tables = hw_specs.get_activation_tables(nc.m.arch)
    if "natural_log_exp_and_others" not in tables:
        return
    for name, funcs in tables.items():
        if name != "natural_log_exp_and_others":
            funcs.discard(AFT.Exp)
            funcs.discard(AFT.Ln)
            funcs.discard(AFT.Square)


def _build_nc():
    import concourse.bacc as bacc
    import concourse.bass as bass
    import concourse.mybir as mybir
    from concourse.tile import TileContext

    f32 = mybir.dt.float32
    f32r = mybir.dt.float32r
    AFT = mybir.ActivationFunctionType
    ALU = mybir.AluOpType

    mmdt = f32r if _MMDT == "f32r" else f32

    nc = bacc.Bacc(None, target_bir_lowering=False)
    _force_combined_act_table(nc, mybir)

    xn = nc.dram_tensor("xn", [128, NT * XNW], mmdt, kind="ExternalInput")
    xtp = nc.dram_tensor("xtp", [128, NPC], mmdt, kind="ExternalInput")
    ct2 = nc.dram_tensor("ct2", [128, K], mmdt, kind="ExternalInput")
    # packed per-k consts: [cc-1 | s | ln s^2], each [128, K]
    cst = nc.dram_tensor("cst", [128, 3 * K], f32, kind="ExternalInput")
    codes_in = nc.dram_tensor("codes", [K, D], f32, kind="ExternalInput")
    e_out = nc.dram_tensor("E", [K, D], f32, kind="ExternalOutput")

    def fcast(ap):
        # view an mmdt tile as plain f32 for non-matmul consumers
        return ap.bitcast(f32) if mmdt is f32r else ap

    def bcast_t(ap32, count):
        # [128, 32] const -> [128, count, 32] via a stride-0 middle dim
        return bass.AP(
            tensor=ap32.tensor,
            offset=ap32.offset,
            ap=[list(ap32.ap[0]), [0, count], list(ap32.ap[1])],
        )

    def bcast_k(apw, count):
        # [128, w] per-tile scalars -> [128, w, count] via stride-0 inner dim
        return bass.AP(
            tensor=apw.tensor,
            offset=apw.offset,
            ap=[list(apw.ap[0]), list(apw.ap[1]), [0, count]],
        )

    with TileContext(nc) as tc:
        with (
            tc.tile_pool(name="singles", bufs=1) as singles,
            tc.tile_pool(name="data", bufs=SC) as data,
            tc.tile_pool(name="work", bufs=SC) as work,
            tc.tile_pool(name="psum_sc", bufs=SC, space="PSUM") as psum_sc,
            tc.tile_pool(name="psum_ag", bufs=1, space="PSUM") as psum_ag,
        ):
            # ---- tiny consts first (they gate everything) ----
            ct2_sb = singles.tile([128, K], mmdt)
            nc.sync.dma_start(out=ct2_sb, in_=ct2[:, :])
            cst_sb = singles.tile([128, 3 * K], f32)
            nc.sync.dma_start(out=cst_sb, in_=cst[:, :])
            ccb_sb = cst_sb[:, 0:K]
            sclb_sb = cst_sb[:, K : 2 * K]
            lns2_sb = cst_sb[:, 2 * K : 3 * K]

            # ---- data DMAs, in the order compute needs them ----
            xn_chunks = []
            xtp_chunks = []
            for c in range(SC):
                xn_c = data.tile([128, TPS * XNW], mmdt, tag="xn")
                xtp_c = data.tile([128, TPS * 128], mmdt, tag="xtp")
                xn_chunks.append(xn_c)
                xtp_chunks.append(xtp_c)
            # first half of xtp0 lands first so matmuls can start asap
            h = TPS * 128 // 2
            nc.sync.dma_start(out=xtp_chunks[0][:, :h], in_=xtp[:, :h])
            nc.sync.dma_start(out=xtp_chunks[0][:, h:], in_=xtp[:, h : TPS * 128])
            nc.sync.dma_start(out=xn_chunks[0], in_=xn[:, : TPS * XNW])
            nc.sync.dma_start(out=xtp_chunks[1], in_=xtp[:, TPS * 128 : 2 * TPS * 128])
            nc.sync.dma_start(out=xn_chunks[1], in_=xn[:, TPS * XNW : 2 * TPS * XNW])
            codes_sb = singles.tile([K, D], f32)
            nc.sync.dma_start(out=codes_sb, in_=codes_in[:, :])

            psum_ms = psum_ag.tile([K, XNW], f32)
            w_chunks = []

            # ---- phase A: scores + softmax weights, per superchunk ----
            for c in range(SC):
                xn_c = xn_chunks[c]
                xtp_c = xtp_chunks[c]

                # scores: -2 x.c for 8 tiles into one PSUM bank
                ps_c = psum_sc.tile([128, TPS * K], f32)
                for i in range(TPS):
                    nc.tensor.matmul(
                        ps_c[:, i * K : (i + 1) * K],
                        xtp_c[:, i * 128 : (i + 1) * 128],
                        ct2_sb,
                        start=True,
                        stop=True,
                    )

                # xx+1 per (token, tile): square the whole [x|1|0] superchunk
                scr_c = work.tile([128, TPS * XNW], f32, tag="scr")
                nc.scalar.activation(out=scr_c, in_=fcast(xn_c), func=AFT.Square)
                xxp_c = work.tile([128, TPS], f32, tag="xxp")
                nc.vector.tensor_reduce(
                    out=xxp_c,
                    in_=scr_c.rearrange("p (t w) -> p t w", w=XNW),
                    axis=mybir.AxisListType.X,
                    op=ALU.add,
                )

                # ccxx[p,t,k] = (cc_k-1) + (xx+1)[p,t]  (independent of PE)
                ccxx_c = work.tile([128, TPS * K], f32, tag="ccxx")
                ccxx3 = ccxx_c.rearrange("p (t k) -> p t k", k=K)
                nc.vector.tensor_add(ccxx3, bcast_t(ccb_sb, TPS), bcast_k(xxp_c, K))

                # A = l2^2 = -2xc + ccxx
                a_c = work.tile([128, TPS * K], f32, tag="a")
                nc.vector.tensor_add(a_c, ps_c, ccxx_c)

                # logits = l2*s = exp(0.5*(ln(l2^2) + ln(s^2)))
                l_c = work.tile([128, TPS * K], f32, tag="l")
                nc.scalar.activation(out=l_c, in_=a_c, func=AFT.Ln)
                t_c = work.tile([128, TPS * K], f32, tag="t")
                nc.vector.tensor_add(
                    t_c.rearrange("p (t k) -> p t k", k=K),
                    l_c.rearrange("p (t k) -> p t k", k=K),
                    bcast_t(lns2_sb, TPS),
                )
                p_c = work.tile([128, TPS * K], f32, tag="p")
                nc.scalar.activation(out=p_c, in_=t_c, func=AFT.Exp, scale=0.5)
                # EW = exp(logits)
                ew_c = work.tile([128, TPS * K], f32, tag="ew")
                nc.scalar.activation(out=ew_c, in_=p_c, func=AFT.Exp)

                # denominators + reciprocal + normalize
                den_c = work.tile([128, TPS], f32, tag="den")
                nc.vector.tensor_reduce(
                    out=den_c,
                    in_=ew_c.rearrange("p (t k) -> p t k", k=K),
                    axis=mybir.AxisListType.X,
                    op=ALU.add,
                )
                rden_c = work.tile([128, TPS], f32, tag="rden")
                nc.vector.reciprocal(out=rden_c, in_=den_c)

                w_c = work.tile([128, TPS * K], mmdt, tag="w")
                nc.vector.tensor_mul(
                    w_c.rearrange("p (t k) -> p t k", k=K),
                    ew_c.rearrange("p (t k) -> p t k", k=K),
                    bcast_k(rden_c, K),
                )
                w_chunks.append(w_c)

            # ---- phase B: aggregation matmuls ----
            for c in range(SC):
                for i in range(TPS):
                    t = c * TPS + i
                    nc.tensor.matmul(
                        psum_ms,
                        w_chunks[c][:, i * K : (i + 1) * K],
                        xn_chunks[c][:, i * XNW : (i + 1) * XNW],
                        start=(t == 0),
                        stop=(t == NT - 1),
                    )

            # ---- final: E = M - S * codes ----
            s_sb = singles.tile([K, 1], f32)
            nc.vector.tensor_copy(s_sb, psum_ms[:, 128:129])
            tmp = singles.tile([K, D], f32)
            nc.vector.tensor_scalar_mul(tmp, codes_sb, s_sb)
            e_sb = singles.tile([K, D], f32)
            nc.vector.tensor_sub(e_sb, psum_ms[:, 0:128], tmp)
            nc.sync.dma_start(out=e_out[:, :], in_=e_sb)

    nc.compile()
    return nc




def _build_nc_raw():
    """Raw-bass variant: same pipeline as the Tile builder but with
    hand-rolled semaphores and no TileContext, which drops the ~10us
    end-of-kernel drain+barrier teardown and most per-op sync overhead.
    DMA dispatches are split across the two HWDGE queues (sync + scalar)."""
    import concourse.bacc as bacc
    import concourse.bass as bass
    import concourse.mybir as mybir

    f32 = mybir.dt.float32
    f32r = mybir.dt.float32r
    AFT = mybir.ActivationFunctionType
    ALU = mybir.AluOpType

    mmdt = f32r if _MMDT == "f32r" else f32

    nc = bacc.Bacc(None, target_bir_lowering=False)
    _force_combined_act_table(nc, mybir)

    HS = TPS * XNW                 # xn elems per superchunk (1040)
    HT = TPS * 128                 # xtp elems per superchunk (1024)

    xn = nc.dram_tensor("xn", [128, NT * XNW], mmdt, kind="ExternalInput")
    xtp = nc.dram_tensor("xtp", [128, NPC], mmdt, kind="ExternalInput")
    cstr = nc.dram_tensor("cstr", [128, K], mmdt, kind="ExternalInput")   # ct2
    cst = nc.dram_tensor("cst", [128, 2 * K], f32, kind="ExternalInput")  # cc-1|lns2
    codes_in = nc.dram_tensor("codes", [K, D], f32, kind="ExternalInput")
    e_out = nc.dram_tensor("E", [K, D], f32, kind="ExternalOutput")

    sb = lambda name, shape, dt: nc.alloc_sbuf_tensor(name, shape, dt)
    ct2_sb = sb("ct2_sb", [128, K], mmdt)
    cst_sb = sb("cst_sb", [128, 2 * K], f32)
    xtp_sb = sb("xtp_sb", [128, NPC], mmdt)
    xn_sb = sb("xn_sb", [128, NT * XNW], mmdt)
    codes_sb = sb("codes_sb", [K, D], f32)
    scr = [sb(f"scr{c}", [128, HS], f32) for c in range(SC)]
    xxp = [sb(f"xxp{c}", [128, TPS], f32) for c in range(SC)]
    ccxx = [sb(f"ccxx{c}", [128, TPS * K], f32) for c in range(SC)]
    a_t = [sb(f"a{c}", [128, TPS * K], f32) for c in range(SC)]
    l_t = [sb(f"l{c}", [128, TPS * K], f32) for c in range(SC)]
    t_t = [sb(f"t{c}", [128, TPS * K], f32) for c in range(SC)]
    p_t = [sb(f"p{c}", [128, TPS * K], f32) for c in range(SC)]
    ew_t = [sb(f"ew{c}", [128, TPS * K], f32) for c in range(SC)]
    den = [sb(f"den{c}", [128, TPS], f32) for c in range(SC)]
    rden = [sb(f"rden{c}", [128, TPS], f32) for c in range(SC)]
    w_t = [sb(f"w{c}", [128, TPS * K], mmdt) for c in range(SC)]
    s_sb = sb("s_sb", [K, 1], f32)
    tmp_sb = sb("tmp_sb", [K, D], f32)
    e_sb = sb("e_sb", [K, D], f32)

    # full-bank allocations so PE writes and DVE reads never share a bank
    ps = [nc.alloc_psum_tensor(f"ps{c}", [128, 512], f32) for c in range(SC)]
    pms = nc.alloc_psum_tensor("pms", [K, XNW], f32)

    ct2v = ct2_sb[:, :]
    ccbv = cst_sb[:, 0:K]
    lnsv = cst_sb[:, K : 2 * K]

    def fc(ap):
        return ap.bitcast(f32) if mmdt is f32r else ap

    def bct(ap32, count):
        return bass.AP(
            tensor=ap32.tensor,
            offset=ap32.offset,
            ap=[list(ap32.ap[0]), [0, count], list(ap32.ap[1])],
        )

    def bck(apw, count):
        return bass.AP(
            tensor=apw.tensor,
            offset=apw.offset,
            ap=[list(apw.ap[0]), list(apw.ap[1]), [0, count]],
        )

    def t3(ap, k=K):
        return ap.rearrange("p (t k) -> p t k", k=k)

    sQr = nc.alloc_semaphore("sQr")      # ct2 DMA done
    sQc = nc.alloc_semaphore("sQc")      # cst DMA done
    sQ1 = nc.alloc_semaphore("sQ1")      # xtp first half
    sQ2 = nc.alloc_semaphore("sQ2")      # xtp second half
    sQ3 = nc.alloc_semaphore("sQ3")      # xtp sc1
    aQ0 = nc.alloc_semaphore("aQ0")      # xn sc0
    aQ1 = nc.alloc_semaphore("aQ1")      # xn sc1
    aQ2 = nc.alloc_semaphore("aQ2")      # codes
    mmS = nc.alloc_semaphore("mmS")      # PE: score matmuls done per sc
    aggS = nc.alloc_semaphore("aggS")    # PE: aggregation done
    sqS = nc.alloc_semaphore("sqS")      # ACT squares done per sc
    lnN = nc.alloc_semaphore("lnN")      # ACT ln done per sc
    ewN = nc.alloc_semaphore("ewN")      # ACT exp(exp) done per sc
    dvA = nc.alloc_semaphore("dvA")      # DVE A=l2^2 ready per sc
    dvT = nc.alloc_semaphore("dvT")      # DVE t=ln(l2^2 s^2) ready per sc
    wR = nc.alloc_semaphore("wR")        # DVE W ready per sc
    eR = nc.alloc_semaphore("eR")        # E ready in SBUF
    oD = nc.alloc_semaphore("oD")        # output DMA done

    with nc.Block(no_gpsimd_drain=True) as block:

        @block.sync
        def _(sync):
            sync.dma_start(out=ct2_sb[:, :], in_=cstr[:, :]).then_inc(sQr, 16)
            sync.dma_start(out=cst_sb[:, :], in_=cst[:, :]).then_inc(sQc, 16)
            sync.dma_start(out=xtp_sb[:, :HT], in_=xtp[:, :HT]).then_inc(sQ1, 16)
            sync.dma_start(out=xtp_sb[:, HT:], in_=xtp[:, HT:]).then_inc(sQ3, 16)
            sync.wait_ge(eR, 1)
            sync.dma_start(out=e_out[:, :], in_=e_sb[:, :]).then_inc(oD, 16)
            sync.wait_ge(oD, 16)

        @block.scalar
        def _(scalar):
            scalar.dma_start(out=xn_sb[:, :HS], in_=xn[:, :HS]).then_inc(aQ0, 16)
            scalar.dma_start(out=xn_sb[:, HS:], in_=xn[:, HS:]).then_inc(aQ1, 16)
            scalar.dma_start(out=codes_sb[:, :], in_=codes_in[:, :]).then_inc(aQ2, 16)
            scalar.wait_ge(aQ0, 16)
            scalar.activation(out=scr[0][:, :], in_=fc(xn_sb[:, :HS]), func=AFT.Square).then_inc(sqS)
            scalar.wait_ge(aQ1, 16)
            scalar.activation(out=scr[1][:, :], in_=fc(xn_sb[:, HS:]), func=AFT.Square).then_inc(sqS)
            scalar.wait_ge(dvA, 1)
            scalar.activation(out=l_t[0][:, :], in_=a_t[0][:, :], func=AFT.Ln).then_inc(lnN)
            scalar.wait_ge(dvT, 1)
            scalar.activation(out=p_t[0][:, :], in_=t_t[0][:, :], func=AFT.Exp, scale=0.5)
            scalar.drain()
            scalar.activation(out=ew_t[0][:, :], in_=p_t[0][:, :], func=AFT.Exp).then_inc(ewN)
            scalar.wait_ge(dvA, 2)
            scalar.activation(out=l_t[1][:, :], in_=a_t[1][:, :], func=AFT.Ln).then_inc(lnN)
            scalar.wait_ge(dvT, 2)
            scalar.activation(out=p_t[1][:, :], in_=t_t[1][:, :], func=AFT.Exp, scale=0.5)
            scalar.drain()
            scalar.activation(out=ew_t[1][:, :], in_=p_t[1][:, :], func=AFT.Exp).then_inc(ewN)

        @block.tensor
        def _(tensor):
            tensor.wait_ge(sQr, 16)
            tensor.wait_ge(sQ1, 16)
            # sQ2 unused: xtp sc0 arrives as one transfer
            for i in range(TPS // 2):
                nc.tensor.matmul(
                    ps[0][:, i * K : (i + 1) * K],
                    xtp_sb[:, i * 128 : (i + 1) * 128],
                    ct2v, start=True, stop=True,
                )
            for i in range(TPS // 2, TPS):
                nc.tensor.matmul(
                    ps[0][:, i * K : (i + 1) * K],
                    xtp_sb[:, i * 128 : (i + 1) * 128],
                    ct2v, start=True, stop=True,
                ).then_inc(mmS) if i == TPS - 1 else nc.tensor.matmul(
                    ps[0][:, i * K : (i + 1) * K],
                    xtp_sb[:, i * 128 : (i + 1) * 128],
                    ct2v, start=True, stop=True,
                )
            tensor.wait_ge(sQ3, 16)
            for i in range(TPS):
                mm = nc.tensor.matmul(
                    ps[1][:, i * K : (i + 1) * K],
                    xtp_sb[:, HT + i * 128 : HT + (i + 1) * 128],
                    ct2v, start=True, stop=True,
                )
                if i == TPS - 1:
                    mm.then_inc(mmS)
            tensor.wait_ge(wR, 1)
            for i in range(TPS):
                nc.tensor.matmul(
                    pms[:, :],
                    w_t[0][:, i * K : (i + 1) * K],
                    xn_sb[:, i * XNW : (i + 1) * XNW],
                    start=(i == 0), stop=False,
                )
            tensor.wait_ge(wR, 2)
            for i in range(TPS):
                mm = nc.tensor.matmul(
                    pms[:, :],
                    w_t[1][:, i * K : (i + 1) * K],
                    xn_sb[:, HS + i * XNW : HS + (i + 1) * XNW],
                    start=False, stop=(i == TPS - 1),
                )
                if i == TPS - 1:
                    mm.then_inc(aggS)

        @block.vector
        def _(vector):
            AX = mybir.AxisListType.X
            vector.wait_ge(sQc, 16)
            vector.wait_ge(sqS, 1)
            nc.vector.tensor_reduce(
                out=xxp[0][:, :], in_=t3(scr[0][:, :], k=XNW), axis=AX, op=ALU.add
            )
            vector.drain()
            nc.vector.tensor_add(t3(ccxx[0][:, :]), bct(ccbv, TPS), bck(xxp[0][:, :], K))
            vector.drain()
            vector.wait_ge(mmS, 1)
            nc.vector.tensor_add(a_t[0][:, :], ps[0][:, : TPS * K], ccxx[0][:, :]).then_inc(dvA)
            vector.wait_ge(lnN, 1)
            nc.vector.tensor_add(
                t3(t_t[0][:, :]), t3(l_t[0][:, :]), bct(lnsv, TPS)
            ).then_inc(dvT)
            vector.wait_ge(sqS, 2)
            nc.vector.tensor_reduce(
                out=xxp[1][:, :], in_=t3(scr[1][:, :], k=XNW), axis=AX, op=ALU.add
            )
            vector.drain()
            nc.vector.tensor_add(t3(ccxx[1][:, :]), bct(ccbv, TPS), bck(xxp[1][:, :], K))
            vector.drain()
            vector.wait_ge(mmS, 2)
            nc.vector.tensor_add(a_t[1][:, :], ps[1][:, : TPS * K], ccxx[1][:, :]).then_inc(dvA)
            vector.wait_ge(ewN, 1)
            nc.vector.tensor_reduce(
                out=den[0][:, :], in_=t3(ew_t[0][:, :]), axis=AX, op=ALU.add
            )
            vector.drain()
            nc.vector.reciprocal(out=rden[0][:, :], in_=den[0][:, :])
            vector.drain()
            nc.vector.tensor_mul(
                t3(w_t[0][:, :]), t3(ew_t[0][:, :]), bck(rden[0][:, :], K)
            ).then_inc(wR)
            vector.wait_ge(lnN, 2)
            nc.vector.tensor_add(
                t3(t_t[1][:, :]), t3(l_t[1][:, :]), bct(lnsv, TPS)
            ).then_inc(dvT)
            vector.wait_ge(ewN, 2)
            nc.vector.tensor_reduce(
                out=den[1][:, :], in_=t3(ew_t[1][:, :]), axis=AX, op=ALU.add
            )
            vector.drain()
            nc.vector.reciprocal(out=rden[1][:, :], in_=den[1][:, :])
            vector.drain()
            nc.vector.tensor_mul(
                t3(w_t[1][:, :]), t3(ew_t[1][:, :]), bck(rden[1][:, :], K)
            ).then_inc(wR)
            vector.wait_ge(aggS, 1)
            vector.wait_ge(aQ2, 16)
            nc.vector.tensor_copy(s_sb[:, :], pms[:, 128:129])
            vector.drain()
            nc.vector.tensor_scalar_mul(tmp_sb[:, :], codes_sb[:, :], s_sb[:, :])
            vector.drain()
            nc.vector.tensor_sub(e_sb[:, :], pms[:, 0:128], tmp_sb[:, :]).then_inc(eR)

    nc.compile()
    return nc

def _get_nc():
    if "nc" not in _CACHE:
        _CACHE["nc"] = _build_nc_raw() if _RAW else _build_nc()
    return _CACHE["nc"]


def _prep_inputs(x, codes, scale):
    """Build the per-core input maps (all host-side numpy)."""
    x = np.asarray(x, dtype=np.float32).reshape(N, D)
    codes = np.asarray(codes, dtype=np.float32)
    scale = np.asarray(scale, dtype=np.float32)

    ct2 = np.ascontiguousarray(-2.0 * codes.T)                      # [D, K]
    cc = (codes * codes).sum(axis=1).astype(np.float32)             # [K]
    lns2 = 2.0 * np.log(np.maximum(scale, np.float32(1e-30)))
    cst = np.ascontiguousarray(
        np.broadcast_to(
            np.concatenate([cc - 1.0, lns2]).astype(np.float32)[None, :],
            (128, 2 * K),
        )
    )

    in_maps = []
    for core in range(N_CORES):
        xs = x[core * NPC : (core + 1) * NPC]                       # [2048, 128]
        a = xs.reshape(128, NT, D)                                  # [p, t, d]
        xnv = np.concatenate(
            [
                a,
                np.ones((128, NT, 1), dtype=np.float32),
                np.zeros((128, NT, 1), dtype=np.float32),
            ],
            axis=2,
        ).reshape(128, NT * XNW)
        xtpv = np.ascontiguousarray(a.transpose(2, 1, 0)).reshape(128, NPC)
        m = {
            "xn": np.ascontiguousarray(xnv),
            "xtp": xtpv,
            "cst": cst,
            "codes": codes,
        }
        if _RAW:
            m["cstr"] = ct2
        if not _RAW:
            m["ct2"] = ct2
            m["cst"] = np.ascontiguousarray(
                np.broadcast_to(
                    np.concatenate([cc - 1.0, scale, lns2]).astype(np.float32)[
                        None, :
                    ],
                    (128, 3 * K),
                )
            )
        in_maps.append(m)
    return in_maps


def kernel(x, codes, scale):
    from concourse.bass_utils import run_bass_kernel_spmd

    nc = _get_nc()
    in_maps = _prep_inputs(x, codes, scale)
    res = run_bass_kernel_spmd(nc, in_maps, core_ids=list(range(N_CORES)))
    out = np.zeros((K, D), dtype=np.float32)
    for r in res.results:
        out += r["E"]
    return out


# revision 22
# speedup vs baseline: 1.1276x; 1.0183x over previous
"""VQ codebook encoding kernel for Trainium2, sharded over 8 NeuronCores.

Math (per shard of N tokens):
    l2[n,k]  = ||x_n - c_k||            (NOT squared)
    W        = softmax_k(l2 * scale_k)
    E[k,d]   = sum_n W[n,k] * (x[n,d] - c[k,d])
             = (W^T X)[k,d] - S_k * c[k,d],   S_k = sum_n W[n,k]

so we never materialize the (N,K,D) residual tensor.  The N axis is
sharded 8 ways; each core returns a partial (K,D) E which the host sums.

Layout tricks:
- x is fed twice per core: natural [n,D] rows (aggregation matmul; a
  [1,0] pad per 128-token tile makes one matmul yield W^T[X|1|0] =
  [M | S | 0] and keeps free dims even for fp32r) and host-pre-transposed
  [D,n] (score matmuls).  Host transposes are free and kill all on-chip
  transposes.
- ||x_n||^2: square the [x|1|0] tile wholesale (ACT) + segmented reduce
  (DVE); the ones column contributes +1, folded into the (cc-1) const.
- softmax: logits = l2*s = exp(0.5*(ln(l2^2) + ln(s^2))); hw_specs'
  cached activation-table map is seeded so ln/exp/square all resolve to
  the one table set containing all three -> exactly one ACT_TABLE_LOAD.
- matmuls run float32r (single-pass on the PE) -- fp32 is 2-pass.
- per-k constants (cc-1, s, ln s^2) are fed as [128,32] tiles and read
  with stride-0 APs to broadcast across the 16 token-tiles; DVE free-dim
  step-0 broadcasts also splice ||x||^2 and 1/den per token-tile.
- DMA order is chosen so the score-matmul operands land first (HWDGE
  transfers complete in FIFO order).
"""

import sys

if "/opt/trn_rl_repo" not in sys.path:
    sys.path.insert(0, "/opt/trn_rl_repo")

import os as _os

import numpy as np

N_CORES = 8
N, K, D = 16384, 32, 128
NPC = N // N_CORES          # tokens per core = 2048
NT = NPC // 128             # 128-token tiles per core = 16
SC = 2                      # superchunks
TPS = NT // SC              # tiles per superchunk = 8
XNW = 130                   # x tile width incl. ones col + pad (fp32r wants even)

_CACHE = {}

_MMDT = _os.environ.get("KMMDT", "f32r")            # f32r | f32
_RAW = _os.environ.get("KRAW", "1") == "1"          # raw bass vs TileContext


def _force_combined_act_table(nc, mybir):
    """Seed hw_specs' cached activation-table dict so the ln/exp/square
    activations all resolve to the one set that contains all three
    ("natural_log_exp_and_others"), giving a single ACT_TABLE_LOAD instead
    of per-func table thrash.  Only mutates the per-process cache copy;
    set ids stay aligned with the compiler's act_info.json."""
    import concourse.hw_specs as hw_specs

    AFT = mybir.ActivationFunctionType
    tables = hw_specs.get_activation_tables(nc.m.arch)
    if "natural_log_exp_and_others" not in tables:
        return
    for name, funcs in tables.items():
        if name != "natural_log_exp_and_others":
            funcs.discard(AFT.Exp)
            funcs.discard(AFT.Ln)
            funcs.discard(AFT.Square)


def _build_nc():
    import concourse.bacc as bacc
    import concourse.bass as bass
    import concourse.mybir as mybir
    from concourse.tile import TileContext

    f32 = mybir.dt.float32
    f32r = mybir.dt.float32r
    AFT = mybir.ActivationFunctionType
    ALU = mybir.AluOpType

    mmdt = f32r if _MMDT == "f32r" else f32

    nc = bacc.Bacc(None, target_bir_lowering=False)
    _force_combined_act_table(nc, mybir)

    xn = nc.dram_tensor("xn", [128, NT * XNW], mmdt, kind="ExternalInput")
    xtp = nc.dram_tensor("xtp", [128, NPC], mmdt, kind="ExternalInput")
    ct2 = nc.dram_tensor("ct2", [128, K], mmdt, kind="ExternalInput")
    # packed per-k consts: [cc-1 | s | ln s^2], each [128, K]
    cst = nc.dram_tensor("cst", [128, 3 * K], f32, kind="ExternalInput")
    codes_in = nc.dram_tensor("codes", [K, D], f32, kind="ExternalInput")
    e_out = nc.dram_tensor("E", [K, D], f32, kind="ExternalOutput")

    def fcast(ap):
        # view an mmdt tile as plain f32 for non-matmul consumers
        return ap.bitcast(f32) if mmdt is f32r else ap

    def bcast_t(ap32, count):
        # [128, 32] const -> [128, count, 32] via a stride-0 middle dim
        return bass.AP(
            tensor=ap32.tensor,
            offset=ap32.offset,
            ap=[list(ap32.ap[0]), [0, count], list(ap32.ap[1])],
        )

    def bcast_k(apw, count):
        # [128, w] per-tile scalars -> [128, w, count] via stride-0 inner dim
        return bass.AP(
            tensor=apw.tensor,
            offset=apw.offset,
            ap=[list(apw.ap[0]), list(apw.ap[1]), [0, count]],
        )

    with TileContext(nc) as tc:
        with (
            tc.tile_pool(name="singles", bufs=1) as singles,
            tc.tile_pool(name="data", bufs=SC) as data,
            tc.tile_pool(name="work", bufs=SC) as work,
            tc.tile_pool(name="psum_sc", bufs=SC, space="PSUM") as psum_sc,
            tc.tile_pool(name="psum_ag", bufs=1, space="PSUM") as psum_ag,
        ):
            # ---- tiny consts first (they gate everything) ----
            ct2_sb = singles.tile([128, K], mmdt)
            nc.sync.dma_start(out=ct2_sb, in_=ct2[:, :])
            cst_sb = singles.tile([128, 3 * K], f32)
            nc.sync.dma_start(out=cst_sb, in_=cst[:, :])
            ccb_sb = cst_sb[:, 0:K]
            sclb_sb = cst_sb[:, K : 2 * K]
            lns2_sb = cst_sb[:, 2 * K : 3 * K]

            # ---- data DMAs, in the order compute needs them ----
            xn_chunks = []
            xtp_chunks = []
            for c in range(SC):
                xn_c = data.tile([128, TPS * XNW], mmdt, tag="xn")
                xtp_c = data.tile([128, TPS * 128], mmdt, tag="xtp")
                xn_chunks.append(xn_c)
                xtp_chunks.append(xtp_c)
            # first half of xtp0 lands first so matmuls can start asap
            h = TPS * 128 // 2
            nc.sync.dma_start(out=xtp_chunks[0][:, :h], in_=xtp[:, :h])
            nc.sync.dma_start(out=xtp_chunks[0][:, h:], in_=xtp[:, h : TPS * 128])
            nc.sync.dma_start(out=xn_chunks[0], in_=xn[:, : TPS * XNW])
            nc.sync.dma_start(out=xtp_chunks[1], in_=xtp[:, TPS * 128 : 2 * TPS * 128])
            nc.sync.dma_start(out=xn_chunks[1], in_=xn[:, TPS * XNW : 2 * TPS * XNW])
            codes_sb = singles.tile([K, D], f32)
            nc.sync.dma_start(out=codes_sb, in_=codes_in[:, :])

            psum_ms = psum_ag.tile([K, XNW], f32)
            w_chunks = []

            # ---- phase A: scores + softmax weights, per superchunk ----
            for c in range(SC):
                xn_c = xn_chunks[c]
                xtp_c = xtp_chunks[c]

                # scores: -2 x.c for 8 tiles into one PSUM bank
                ps_c = psum_sc.tile([128, TPS * K], f32)
                for i in range(TPS):
                    nc.tensor.matmul(
                        ps_c[:, i * K : (i + 1) * K],
                        xtp_c[:, i * 128 : (i + 1) * 128],
                        ct2_sb,
                        start=True,
                        stop=True,
                    )

                # xx+1 per (token, tile): square the whole [x|1|0] superchunk
                scr_c = work.tile([128, TPS * XNW], f32, tag="scr")
                nc.scalar.activation(out=scr_c, in_=fcast(xn_c), func=AFT.Square)
                xxp_c = work.tile([128, TPS], f32, tag="xxp")
                nc.vector.tensor_reduce(
                    out=xxp_c,
                    in_=scr_c.rearrange("p (t w) -> p t w", w=XNW),
                    axis=mybir.AxisListType.X,
                    op=ALU.add,
                )

                # ccxx[p,t,k] = (cc_k-1) + (xx+1)[p,t]  (independent of PE)
                ccxx_c = work.tile([128, TPS * K], f32, tag="ccxx")
                ccxx3 = ccxx_c.rearrange("p (t k) -> p t k", k=K)
                nc.vector.tensor_add(ccxx3, bcast_t(ccb_sb, TPS), bcast_k(xxp_c, K))

                # A = l2^2 = -2xc + ccxx
                a_c = work.tile([128, TPS * K], f32, tag="a")
                nc.vector.tensor_add(a_c, ps_c, ccxx_c)

                # logits = l2*s = exp(0.5*(ln(l2^2) + ln(s^2)))
                l_c = work.tile([128, TPS * K], f32, tag="l")
                nc.scalar.activation(out=l_c, in_=a_c, func=AFT.Ln)
                t_c = work.tile([128, TPS * K], f32, tag="t")
                nc.vector.tensor_add(
                    t_c.rearrange("p (t k) -> p t k", k=K),
                    l_c.rearrange("p (t k) -> p t k", k=K),
                    bcast_t(lns2_sb, TPS),
                )
                p_c = work.tile([128, TPS * K], f32, tag="p")
                nc.scalar.activation(out=p_c, in_=t_c, func=AFT.Exp, scale=0.5)
                # EW = exp(logits)
                ew_c = work.tile([128, TPS * K], f32, tag="ew")
                nc.scalar.activation(out=ew_c, in_=p_c, func=AFT.Exp)

                # denominators + reciprocal + normalize
                den_c = work.tile([128, TPS], f32, tag="den")
                nc.vector.tensor_reduce(
                    out=den_c,
                    in_=ew_c.rearrange("p (t k) -> p t k", k=K),
                    axis=mybir.AxisListType.X,
                    op=ALU.add,
                )
                rden_c = work.tile([128, TPS], f32, tag="rden")
                nc.vector.reciprocal(out=rden_c, in_=den_c)

                w_c = work.tile([128, TPS * K], mmdt, tag="w")
                nc.vector.tensor_mul(
                    w_c.rearrange("p (t k) -> p t k", k=K),
                    ew_c.rearrange("p (t k) -> p t k", k=K),
                    bcast_k(rden_c, K),
                )
                w_chunks.append(w_c)

            # ---- phase B: aggregation matmuls ----
            for c in range(SC):
                for i in range(TPS):
                    t = c * TPS + i
                    nc.tensor.matmul(
                        psum_ms,
                        w_chunks[c][:, i * K : (i + 1) * K],
                        xn_chunks[c][:, i * XNW : (i + 1) * XNW],
                        start=(t == 0),
                        stop=(t == NT - 1),
                    )

            # ---- final: E = M - S * codes ----
            s_sb = singles.tile([K, 1], f32)
            nc.vector.tensor_copy(s_sb, psum_ms[:, 128:129])
            tmp = singles.tile([K, D], f32)
            nc.vector.tensor_scalar_mul(tmp, codes_sb, s_sb)
            e_sb = singles.tile([K, D], f32)
            nc.vector.tensor_sub(e_sb, psum_ms[:, 0:128], tmp)
            nc.sync.dma_start(out=e_out[:, :], in_=e_sb)

    nc.compile()
    return nc




def _build_nc_raw():
    """Raw-bass variant: same pipeline as the Tile builder but with
    hand-rolled semaphores and no TileContext, which drops the ~10us
    end-of-kernel drain+barrier teardown and most per-op sync overhead.
    DMA dispatches are split across the two HWDGE queues (sync + scalar)."""
    import concourse.bacc as bacc
    import concourse.bass as bass
    import concourse.mybir as mybir

    f32 = mybir.dt.float32
    f32r = mybir.dt.float32r
    AFT = mybir.ActivationFunctionType
    ALU = mybir.AluOpType

    mmdt = f32r if _MMDT == "f32r" else f32

    nc = bacc.Bacc(None, target_bir_lowering=False)
    _force_combined_act_table(nc, mybir)

    HS = TPS * XNW                 # xn elems per superchunk (1040)
    HT = TPS * 128                 # xtp elems per superchunk (1024)

    xn = nc.dram_tensor("xn", [128, NT * XNW], mmdt, kind="ExternalInput")
    xtp = nc.dram_tensor("xtp", [128, NPC], mmdt, kind="ExternalInput")
    cstr = nc.dram_tensor("cstr", [128, K], mmdt, kind="ExternalInput")   # ct2
    cst = nc.dram_tensor("cst", [128, 2 * K], f32, kind="ExternalInput")  # cc-1|lns2
    codes_in = nc.dram_tensor("codes", [K, D], f32, kind="ExternalInput")
    e_out = nc.dram_tensor("E", [K, D], f32, kind="ExternalOutput")

    sb = lambda name, shape, dt: nc.alloc_sbuf_tensor(name, shape, dt)
    ct2_sb = sb("ct2_sb", [128, K], mmdt)
    cst_sb = sb("cst_sb", [128, 2 * K], f32)
    xtp_sb = sb("xtp_sb", [128, NPC], mmdt)
    xn_sb = sb("xn_sb", [128, NT * XNW], mmdt)
    codes_sb = sb("codes_sb", [K, D], f32)
    scr = [sb(f"scr{c}", [128, HS], f32) for c in range(SC)]
    xxp = [sb(f"xxp{c}", [128, TPS], f32) for c in range(SC)]
    ccxx = [sb(f"ccxx{c}", [128, TPS * K], f32) for c in range(SC)]
    a_t = [sb(f"a{c}", [128, TPS * K], f32) for c in range(SC)]
    l_t = [sb(f"l{c}", [128, TPS * K], f32) for c in range(SC)]
    t_t = [sb(f"t{c}", [128, TPS * K], f32) for c in range(SC)]
    p_t = [sb(f"p{c}", [128, TPS * K], f32) for c in range(SC)]
    ew_t = [sb(f"ew{c}", [128, TPS * K], f32) for c in range(SC)]
    den = [sb(f"den{c}", [128, TPS], f32) for c in range(SC)]
    rden = [sb(f"rden{c}", [128, TPS], f32) for c in range(SC)]
    w_t = [sb(f"w{c}", [128, TPS * K], mmdt) for c in range(SC)]
    s_sb = sb("s_sb", [K, 1], f32)
    tmp_sb = sb("tmp_sb", [K, D], f32)
    e_sb = sb("e_sb", [K, D], f32)

    # full-bank allocations so PE writes and DVE reads never share a bank
    ps = [nc.alloc_psum_tensor(f"ps{c}", [128, 512], f32) for c in range(SC)]
    pms = nc.alloc_psum_tensor("pms", [K, XNW], f32)

    ct2v = ct2_sb[:, :]
    ccbv = cst_sb[:, 0:K]
    lnsv = cst_sb[:, K : 2 * K]

    def fc(ap):
        return ap.bitcast(f32) if mmdt is f32r else ap

    def bct(ap32, count):
        return bass.AP(
            tensor=ap32.tensor,
            offset=ap32.offset,
            ap=[list(ap32.ap[0]), [0, count], list(ap32.ap[1])],
        )

    def bck(apw, count):
        return bass.AP(
            tensor=apw.tensor,
            offset=apw.offset,
            ap=[list(apw.ap[0]), list(apw.ap[1]), [0, count]],
        )

    def t3(ap, k=K):
        return ap.rearrange("p (t k) -> p t k", k=k)

    sQr = nc.alloc_semaphore("sQr")      # ct2 DMA done
    sQc = nc.alloc_semaphore("sQc")      # cst DMA done
    sQ1 = nc.alloc_semaphore("sQ1")      # xtp first half
    sQ2 = nc.alloc_semaphore("sQ2")      # xtp second half
    sQ3 = nc.alloc_semaphore("sQ3")      # xtp sc1
    aQ0 = nc.alloc_semaphore("aQ0")      # xn sc0
    aQ1 = nc.alloc_semaphore("aQ1")      # xn sc1
    aQ2 = nc.alloc_semaphore("aQ2")      # codes
    mmS = nc.alloc_semaphore("mmS")      # PE: score matmuls done per sc
    aggS = nc.alloc_semaphore("aggS")    # PE: aggregation done
    sqS = nc.alloc_semaphore("sqS")      # ACT squares done per sc
    lnN = nc.alloc_semaphore("lnN")      # ACT ln done per sc
    ewN = nc.alloc_semaphore("ewN")      # ACT exp(exp) done per sc
    dvA = nc.alloc_semaphore("dvA")      # DVE A=l2^2 ready per sc
    dvT = nc.alloc_semaphore("dvT")      # DVE t=ln(l2^2 s^2) ready per sc
    wR = nc.alloc_semaphore("wR")        # DVE W ready per sc
    eR = nc.alloc_semaphore("eR")        # E ready in SBUF
    oD = nc.alloc_semaphore("oD")        # output DMA done

    with nc.Block(no_gpsimd_drain=True) as block:

        @block.sync
        def _(sync):
            sync.dma_start(out=ct2_sb[:, :], in_=cstr[:, :]).then_inc(sQr, 16)
            sync.dma_start(out=cst_sb[:, :], in_=cst[:, :]).then_inc(sQc, 16)
            h = HT // 2
            sync.dma_start(out=xtp_sb[:, :h], in_=xtp[:, :h]).then_inc(sQ1, 16)
            sync.dma_start(out=xtp_sb[:, h:HT], in_=xtp[:, h:HT]).then_inc(sQ2, 16)
            sync.dma_start(out=xtp_sb[:, HT:], in_=xtp[:, HT:]).then_inc(sQ3, 16)
            sync.wait_ge(eR, 1)
            sync.dma_start(out=e_out[:, :], in_=e_sb[:, :]).then_inc(oD, 16)
            sync.wait_ge(oD, 16)

        @block.scalar
        def _(scalar):
            scalar.dma_start(out=xn_sb[:, :HS], in_=xn[:, :HS]).then_inc(aQ0, 16)
            scalar.dma_start(out=xn_sb[:, HS:], in_=xn[:, HS:]).then_inc(aQ1, 16)
            scalar.dma_start(out=codes_sb[:, :], in_=codes_in[:, :]).then_inc(aQ2, 16)
            scalar.wait_ge(aQ0, 16)
            scalar.activation(out=scr[0][:, :], in_=fc(xn_sb[:, :HS]), func=AFT.Square).then_inc(sqS)
            scalar.wait_ge(aQ1, 16)
            scalar.activation(out=scr[1][:, :], in_=fc(xn_sb[:, HS:]), func=AFT.Square).then_inc(sqS)
            scalar.wait_ge(dvA, 1)
            scalar.activation(out=l_t[0][:, :], in_=a_t[0][:, :], func=AFT.Ln).then_inc(lnN)
            scalar.wait_ge(dvT, 1)
            scalar.activation(out=p_t[0][:, :], in_=t_t[0][:, :], func=AFT.Exp, scale=0.5)
            scalar.drain()
            scalar.activation(out=ew_t[0][:, :], in_=p_t[0][:, :], func=AFT.Exp).then_inc(ewN)
            scalar.wait_ge(dvA, 2)
            scalar.activation(out=l_t[1][:, :], in_=a_t[1][:, :], func=AFT.Ln).then_inc(lnN)
            scalar.wait_ge(dvT, 2)
            scalar.activation(out=p_t[1][:, :], in_=t_t[1][:, :], func=AFT.Exp, scale=0.5)
            scalar.drain()
            scalar.activation(out=ew_t[1][:, :], in_=p_t[1][:, :], func=AFT.Exp).then_inc(ewN)

        @block.tensor
        def _(tensor):
            tensor.wait_ge(sQr, 16)
            tensor.wait_ge(sQ1, 16)
            for i in range(TPS // 2):
                nc.tensor.matmul(
                    ps[0][:, i * K : (i + 1) * K],
                    xtp_sb[:, i * 128 : (i + 1) * 128],
                    ct2v, start=True, stop=True,
                )
            tensor.wait_ge(sQ2, 16)
            for i in range(TPS // 2, TPS):
                nc.tensor.matmul(
                    ps[0][:, i * K : (i + 1) * K],
                    xtp_sb[:, i * 128 : (i + 1) * 128],
                    ct2v, start=True, stop=True,
                ).then_inc(mmS) if i == TPS - 1 else nc.tensor.matmul(
                    ps[0][:, i * K : (i + 1) * K],
                    xtp_sb[:, i * 128 : (i + 1) * 128],
                    ct2v, start=True, stop=True,
                )
            tensor.wait_ge(sQ3, 16)
            for i in range(TPS):
                mm = nc.tensor.matmul(
                    ps[1][:, i * K : (i + 1) * K],
                    xtp_sb[:, HT + i * 128 : HT + (i + 1) * 128],
                    ct2v, start=True, stop=True,
                )
                if i == TPS - 1:
                    mm.then_inc(mmS)
            tensor.wait_ge(wR, 1)
            for i in range(TPS):
                nc.tensor.matmul(
                    pms[:, :],
                    w_t[0][:, i * K : (i + 1) * K],
                    xn_sb[:, i * XNW : (i + 1) * XNW],
                    start=(i == 0), stop=False,
                )
            tensor.wait_ge(wR, 2)
            for i in range(TPS):
                mm = nc.tensor.matmul(
                    pms[:, :],
                    w_t[1][:, i * K : (i + 1) * K],
                    xn_sb[:, HS + i * XNW : HS + (i + 1) * XNW],
                    start=False, stop=(i == TPS - 1),
                )
                if i == TPS - 1:
                    mm.then_inc(aggS)

        @block.vector
        def _(vector):
            AX = mybir.AxisListType.X
            vector.wait_ge(sQc, 16)
            vector.wait_ge(sqS, 1)
            nc.vector.tensor_reduce(
                out=xxp[0][:, :], in_=t3(scr[0][:, :], k=XNW), axis=AX, op=ALU.add
            )
            vector.drain()
            nc.vector.tensor_add(t3(ccxx[0][:, :]), bct(ccbv, TPS), bck(xxp[0][:, :], K))
            vector.drain()
            vector.wait_ge(mmS, 1)
            nc.vector.tensor_add(a_t[0][:, :], ps[0][:, : TPS * K], ccxx[0][:, :]).then_inc(dvA)
            vector.wait_ge(lnN, 1)
            nc.vector.tensor_add(
                t3(t_t[0][:, :]), t3(l_t[0][:, :]), bct(lnsv, TPS)
            ).then_inc(dvT)
            vector.wait_ge(sqS, 2)
            nc.vector.tensor_reduce(
                out=xxp[1][:, :], in_=t3(scr[1][:, :], k=XNW), axis=AX, op=ALU.add
            )
            vector.drain()
            nc.vector.tensor_add(t3(ccxx[1][:, :]), bct(ccbv, TPS), bck(xxp[1][:, :], K))
            vector.drain()
            vector.wait_ge(mmS, 2)
            nc.vector.tensor_add(a_t[1][:, :], ps[1][:, : TPS * K], ccxx[1][:, :]).then_inc(dvA)
            vector.wait_ge(ewN, 1)
            nc.vector.tensor_reduce(
                out=den[0][:, :], in_=t3(ew_t[0][:, :]), axis=AX, op=ALU.add
            )
            vector.drain()
            nc.vector.reciprocal(out=rden[0][:, :], in_=den[0][:, :])
            vector.drain()
            nc.vector.tensor_mul(
                t3(w_t[0][:, :]), t3(ew_t[0][:, :]), bck(rden[0][:, :], K)
            ).then_inc(wR)
            vector.wait_ge(lnN, 2)
            nc.vector.tensor_add(
                t3(t_t[1][:, :]), t3(l_t[1][:, :]), bct(lnsv, TPS)
            ).then_inc(dvT)
            vector.wait_ge(ewN, 2)
            nc.vector.tensor_reduce(
                out=den[1][:, :], in_=t3(ew_t[1][:, :]), axis=AX, op=ALU.add
            )
            vector.drain()
            nc.vector.reciprocal(out=rden[1][:, :], in_=den[1][:, :])
            vector.drain()
            nc.vector.tensor_mul(
                t3(w_t[1][:, :]), t3(ew_t[1][:, :]), bck(rden[1][:, :], K)
            ).then_inc(wR)
            vector.wait_ge(aggS, 1)
            vector.wait_ge(aQ2, 16)
            nc.vector.tensor_copy(s_sb[:, :], pms[:, 128:129])
            vector.drain()
            nc.vector.tensor_scalar_mul(tmp_sb[:, :], codes_sb[:, :], s_sb[:, :])
            vector.drain()
            nc.vector.tensor_sub(e_sb[:, :], pms[:, 0:128], tmp_sb[:, :]).then_inc(eR)

    nc.compile()
    return nc

def _get_nc():
    if "nc" not in _CACHE:
        _CACHE["nc"] = _build_nc_raw() if _RAW else _build_nc()
    return _CACHE["nc"]


def _prep_inputs(x, codes, scale):
    """Build the per-core input maps (all host-side numpy)."""
    x = np.asarray(x, dtype=np.float32).reshape(N, D)
    codes = np.asarray(codes, dtype=np.float32)
    scale = np.asarray(scale, dtype=np.float32)

    ct2 = np.ascontiguousarray(-2.0 * codes.T)                      # [D, K]
    cc = (codes * codes).sum(axis=1).astype(np.float32)             # [K]
    lns2 = 2.0 * np.log(np.maximum(scale, np.float32(1e-30)))
    cst = np.ascontiguousarray(
        np.broadcast_to(
            np.concatenate([cc - 1.0, lns2]).astype(np.float32)[None, :],
            (128, 2 * K),
        )
    )

    in_maps = []
    for core in range(N_CORES):
        xs = x[core * NPC : (core + 1) * NPC]                       # [2048, 128]
        a = xs.reshape(128, NT, D)                                  # [p, t, d]
        xnv = np.concatenate(
            [
                a,
                np.ones((128, NT, 1), dtype=np.float32),
                np.zeros((128, NT, 1), dtype=np.float32),
            ],
            axis=2,
        ).reshape(128, NT * XNW)
        xtpv = np.ascontiguousarray(a.transpose(2, 1, 0)).reshape(128, NPC)
        m = {
            "xn": np.ascontiguousarray(xnv),
            "xtp": xtpv,
            "cst": cst,
            "codes": codes,
        }
        if _RAW:
            m["cstr"] = ct2
        if not _RAW:
            m["ct2"] = ct2
            m["cst"] = np.ascontiguousarray(
                np.broadcast_to(
                    np.concatenate([cc - 1.0, scale, lns2]).astype(np.float32)[
                        None, :
                    ],
                    (128, 3 * K),
                )
            )
        in_maps.append(m)
    return in_maps


def kernel(x, codes, scale):
    from concourse.bass_utils import run_bass_kernel_spmd

    nc = _get_nc()
    in_maps = _prep_inputs(x, codes, scale)
    res = run_bass_kernel_spmd(nc, in_maps, core_ids=list(range(N_CORES)))
    out = np.zeros((K, D), dtype=np.float32)
    for r in res.results:
        out += r["E"]
    return out
